# revision 24
# baseline (speedup 1.0000x reference)
"""DeepSeek sparse attention (lightning indexer + top-64) on 8 trn2 cores.

Strategy (fully static SPMD program; per-core variation is data-only):
  - Each core owns 4 query tiles of 128 queries ("slots" with fixed key-widths
    512/1024/1536/2048).  Slot -> (batch, qtile) assignment is done on the
    host; each core uploads xT for the two batches it touches ("P" side full
    2048 cols, "Q" side first 1536 cols), the 512 query columns, and an
    additive causal/pad mask per slot.
  - Indexer path (projections + qI.kI scores) runs in fp32 (float32r matmuls)
    so the top-64 selection matches the fp32 reference bit-nearly-exactly.
  - Top-64 per query via 8 rounds of max8 + match_replace; selection becomes
    a 0/1 mask (score > zapped) and routing weights W = score * mask.
  - Attention avoids any gather: logits are computed densely in BOTH
    orientations.  [key, query] orientation: P~T = exp(L/8) * W^T feeds PV
    matmuls directly (masked positions have W == 0).  [query, key]
    orientation feeds exp + masked row-sum (tensor_tensor_reduce) to get the
    softmax denominator.  1/denominator is applied on the tiny PV output.
  - Output projection consumes the feature-major attention output, yielding
    token-major [512, 768] per core, DMA'd straight from PSUM.
"""

import numpy as np
import ml_dtypes

import concourse.bass as bass
import concourse.bacc as bacc
import concourse.mybir as mybir
import concourse.tile as tile
from concourse.bass_utils import run_bass_kernel_spmd

f32 = mybir.dt.float32
f32r = mybir.dt.float32r
bf16 = mybir.dt.bfloat16
AL = mybir.AluOpType
AF = mybir.ActivationFunctionType
AX = mybir.AxisListType

FMIN = -3.0e38
B, T, D = 2, 2048, 768
H, DH, IH, ID, TOPK = 12, 64, 4, 64, 64
TP, TQ = 2048, 1536          # key extents kept for the two batch "sides"
SLOT_W = (512, 1024, 1536, 2048)
SOFF = (0, 512, 1536, 3072)  # smask column offset per slot
SMW = 5120
NCORES = 8


def slot_map(c):
    """slot j -> (batch, qtile_index, side)   side 0 = "P", 1 = "Q"."""
    d, p = c // 2, c % 2
    q = 1 - p
    return [(p, d, 0), (q, 7 - d, 1), (q, 8 + d, 1), (p, 15 - d, 0)]


def _r(ap):
    """[768, C] dram/sbuf view -> [128, 6, C]."""
    return ap.rearrange("(a p) c -> p a c", p=128)


def build_program(signs):
    import os
    _ = os.environ.get("KSTAGE", "full")
    nc = bacc.Bacc("TRN2", target_bir_lowering=False, debug=False,
                   num_devices=NCORES)

    xT_P = nc.dram_tensor("xT_P", [D, TP], f32, kind="ExternalInput").ap()
    xT_Q = nc.dram_tensor("xT_Q", [D, TQ], f32, kind="ExternalInput").ap()
    xTq = nc.dram_tensor("xTq", [D, 512], f32, kind="ExternalInput").ap()
    wqkvT = nc.dram_tensor("wqkvT", [D, 3 * D], bf16, kind="ExternalInput").ap()
    wiqT = nc.dram_tensor("wiqT", [D, IH * ID], f32, kind="ExternalInput").ap()
    wikT = nc.dram_tensor("wikT", [D, IH * ID], f32, kind="ExternalInput").ap()
    woutT = nc.dram_tensor("woutT", [D, D], bf16, kind="ExternalInput").ap()
    smask = nc.dram_tensor("smask", [128, SMW], bf16, kind="ExternalInput").ap()
    out_d = nc.dram_tensor("out", [512, D], f32, kind="ExternalOutput").ap()

    with tile.TileContext(nc) as tc:
        _body(tc, xT_P, xT_Q, xTq, wqkvT, wiqT, wikT, woutT, smask, out_d,
              signs)
    nc.compile()
    return nc


def _body(tc, xT_P, xT_Q, xTq, wqkvT, wiqT, wikT, woutT, smask, out_d, signs):
    nc = tc.nc
    import os
    STAGE = os.environ.get("KSTAGE", "full")

    # ---------------- persistent tensors ----------------
    from contextlib import ExitStack
    _px = ExitStack()
    pp = _px.enter_context(tc.tile_pool(name="persist", bufs=1))
    KT_P = pp.tile([128, 6, TP], bf16, name="KT_P")
    KT_Q = pp.tile([128, 6, TQ], bf16, name="KT_Q")
    V_P = pp.tile([128, TP // 128, D], bf16, name="V_P")
    V_Q = pp.tile([128, TQ // 128, D], bf16, name="V_Q")
    kIT_P = pp.tile([128, 2, TP], f32r, name="kIT_P")
    kIT_Q = pp.tile([128, 2, TQ], f32r, name="kIT_Q")
    QT = pp.tile([128, 6, 512], bf16, name="QT")
    qIT = pp.tile([128, 2, 512], f32r, name="qIT")
    otsb = pp.tile([128, 6, 512], bf16, name="otsb")
    id_bf = pp.tile([128, 128], bf16, name="id_bf")
    ones_bf = pp.tile([128, 1], bf16, name="ones_bf")

    # identity matrix for PE transposes
    nc.vector.memset(id_bf, 1.0)
    nc.gpsimd.affine_select(id_bf, id_bf, pattern=[[-1, 128]],
                            compare_op=AL.is_equal, fill=0.0, base=0,
                            channel_multiplier=1)
    nc.vector.memset(ones_bf, 1.0)

    # ---------------- stage 1: projections ----------------
    CH = 256
    with tc.tile_pool(name="w1", bufs=1) as wp, \
         tc.tile_pool(name="ps1", bufs=3, space="PSUM") as ps1, \
         tc.tile_pool(name="psv", bufs=2, space="PSUM") as psv:
        wqkv_sb = wp.tile([128, 6, 3 * D], bf16)
        wiq_sb = wp.tile([128, 6, IH * ID], f32)
        wik_sb = wp.tile([128, 6, IH * ID], f32)
        nc.sync.dma_start(out=wqkv_sb, in_=_r(wqkvT))
        nc.sync.dma_start(out=wiq_sb, in_=_r(wiqT))
        nc.sync.dma_start(out=wik_sb, in_=_r(wikT))
        # fp32r operands must come from an op that rounds to fp32r
        wiq_r = wp.tile([128, 6, IH * ID], f32r)
        wik_r = wp.tile([128, 6, IH * ID], f32r)
        nc.any.tensor_copy(wiq_r.rearrange("p a c -> p (a c)"),
                           wiq_sb.rearrange("p a c -> p (a c)"))
        nc.any.tensor_copy(wik_r.rearrange("p a c -> p (a c)"),
                           wik_sb.rearrange("p a c -> p (a c)"))

        # queries first (they are needed by every slot)
        with tc.tile_pool(name="xq", bufs=1) as xqp:
            xqv = _r(xTq)
            for q0 in range(0, 512, 256):
                xq = xqp.tile([128, 6, 256], f32, name="xq", tag="xq")
                nc.sync.dma_start(out=xq, in_=xqv[:, :, q0:q0 + 256])
                xqb = xqp.tile([128, 6, 256], bf16, name="xqb", tag="xqb")
                nc.any.tensor_copy(xqb.rearrange("p a c -> p (a c)"),
                                   xq.rearrange("p a c -> p (a c)"))
                xqr = xqp.tile([128, 6, 256], f32r, name="xqr", tag="xqr")
                nc.gpsimd.tensor_copy(xqr.rearrange("p a c -> p (a c)"),
                                      xq.rearrange("p a c -> p (a c)"))
                for mt in range(2):
                    ps = ps1.tile([128, 256], f32, name="ps_qi", tag="ps1")
                    for kt in range(6):
                        nc.tensor.matmul(
                            ps, lhsT=wiq_r[:, kt, mt * 128:(mt + 1) * 128],
                            rhs=xqr[:, kt, :], start=(kt == 0), stop=(kt == 5))
                    nc.any.tensor_copy(qIT[:, mt, q0:q0 + 256], ps)
                for mt in range(6):
                    ps = ps1.tile([128, 256], f32, name="ps_qt", tag="ps1")
                    for kt in range(6):
                        nc.tensor.matmul(
                            ps, lhsT=wqkv_sb[:, kt, mt * 128:(mt + 1) * 128],
                            rhs=xqb[:, kt, :], start=(kt == 0), stop=(kt == 5))
                    nc.any.tensor_copy(QT[:, mt, q0:q0 + 256], ps)

        with tc.tile_pool(name="xc", bufs=2) as xcp, \
             tc.tile_pool(name="xcb", bufs=2) as xbp:
            for side, (xT, S) in enumerate(((xT_P, TP), (xT_Q, TQ))):
                xv = _r(xT)
                KT = KT_P if side == 0 else KT_Q
                Vt = V_P if side == 0 else V_Q
                kIT = kIT_P if side == 0 else kIT_Q
                for c0 in range(0, S, CH):
                    xc = xcp.tile([128, 6, CH], f32, name="xc", tag="xc")
                    nc.sync.dma_start(out=xc, in_=xv[:, :, c0:c0 + CH])
                    xcb = xbp.tile([128, 6, CH], bf16, name="xcb", tag="xcb")
                    nc.any.tensor_copy(xcb.rearrange("p a c -> p (a c)"),
                                       xc.rearrange("p a c -> p (a c)"))
                    xcr = xbp.tile([128, 6, CH], f32r, name="xcr", tag="xcr", bufs=1)
                    nc.gpsimd.tensor_copy(xcr.rearrange("p a c -> p (a c)"),
                                          xc.rearrange("p a c -> p (a c)"))
                    # indexer keys (fp32r, exact)
                    for mt in range(2):
                        ps = ps1.tile([128, CH], f32, name="ps_ki", tag="ps1")
                        for kt in range(6):
                            nc.tensor.matmul(
                                ps, lhsT=wik_r[:, kt, mt * 128:(mt + 1) * 128],
                                rhs=xcr[:, kt, :],
                                start=(kt == 0), stop=(kt == 5))
                        nc.any.tensor_copy(kIT[:, mt, c0:c0 + CH], ps)
                    # attention keys KT (bf16)
                    for mt in range(6):
                        ps = ps1.tile([128, CH], f32, name="ps_kt", tag="ps1")
                        for kt in range(6):
                            nc.tensor.matmul(
                                ps,
                                lhsT=wqkv_sb[:, kt,
                                             D + mt * 128:D + (mt + 1) * 128],
                                rhs=xcb[:, kt, :], start=(kt == 0),
                                stop=(kt == 5))
                        nc.any.tensor_copy(KT[:, mt, c0:c0 + CH], ps)
                    # values V (token-major, bf16)
                    for st in range(CH // 128):
                        ps = psv.tile([128, D], f32, name="ps_v", tag="psv")
                        for n0, nn in ((0, 512), (512, 256)):
                            for kt in range(6):
                                nc.tensor.matmul(
                                    ps[:, n0:n0 + nn],
                                    lhsT=xcb[:, kt, st * 128:(st + 1) * 128],
                                    rhs=wqkv_sb[:, kt,
                                                2 * D + n0:2 * D + n0 + nn],
                                    start=(kt == 0), stop=(kt == 5))
                        nc.any.tensor_copy(Vt[:, c0 // 128 + st, :], ps)

    # ---------------- stage 2/3: per query tile ----------------
    with tc.tile_pool(name="w2", bufs=1) as wp2, \
         tc.tile_pool(name="sc", bufs=2) as scp, \
         tc.tile_pool(name="zap", bufs=1) as zpp, \
         tc.tile_pool(name="msk", bufs=2) as mkp, \
         tc.tile_pool(name="wts", bufs=2) as wtp, \
         tc.tile_pool(name="u", bufs=2) as upl, \
         tc.tile_pool(name="et", bufs=2 if STAGE == "D" else 3) as etp, \
         tc.tile_pool(name="pt", bufs=3) as ptp, \
         tc.tile_pool(name="sm", bufs=4) as smp, \
         tc.tile_pool(name="rp", bufs=1) as rpp, \
         tc.tile_pool(name="ps2", bufs=2, space="PSUM") as ps2, \
         tc.tile_pool(name="psot", bufs=3, space="PSUM") as psot, \
         tc.tile_pool(name="psD", bufs=2, space="PSUM") as psD, \
         tc.tile_pool(name="ddr", bufs=2, space="DRAM") as ddr:
        wout_sb = wp2.tile([128, 6, D], bf16)
        nc.sync.dma_start(out=wout_sb, in_=_r(woutT))
        smask_sb = wp2.tile([128, SMW], bf16)
        nc.sync.dma_start(out=smask_sb, in_=smask)

        for j in (0, 3, 2, 1):
            W = SLOT_W[j]
            side = (0, 1, 1, 0)[j]
            KT = KT_P if side == 0 else KT_Q
            Vt = V_P if side == 0 else V_Q
            kIT = kIT_P if side == 0 else kIT_Q
            NT = W // 128
            NCH = W // 512
            qs = slice(j * 128, (j + 1) * 128)

            # ---- indexer scores ----
            scores = scp.tile([128, 2048], f32, name="scores", tag="scores")
            for ch in range(NCH):
                cs = slice(ch * 512, (ch + 1) * 512)
                us = []
                for h4 in range(IH):
                    hp, t2 = h4 % 2, h4 // 2
                    ps = ps2.tile([128, 512], f32, name="ps_ix", tag="ps2")
                    nc.tensor.matmul(
                        ps, lhsT=qIT[64 * hp:64 * hp + 64, t2, qs],
                        rhs=kIT[64 * hp:64 * hp + 64, t2, cs])
                    uh = upl.tile([128, 512], f32, name="u", tag="u")
                    nc.scalar.activation(uh, ps, AF.Relu)
                    us.append(uh)
                # signed head-sum + causal/pad mask, all on gpsimd:
                # sc = smask +- u0 +- u1 +- u2 +- u3  (|w_h| folded into qIT)
                smk = smask_sb[:, SOFF[j] + ch * 512:SOFF[j] + (ch + 1) * 512]
                sc = scores[:, cs]
                if signs[0] > 0:
                    nc.gpsimd.tensor_tensor(sc, us[0], smk, AL.add)
                else:
                    nc.gpsimd.tensor_tensor(sc, smk, us[0], AL.subtract)
                for h4 in range(1, IH):
                    nc.gpsimd.tensor_tensor(
                        sc, sc, us[h4],
                        AL.add if signs[h4] > 0 else AL.subtract)

            # ---- top-64 threshold via 8x (max8 + match_replace) ----
            zap = zpp.tile([128, 2048], f32, name="zap", tag="zap")
            src = scores
            for r in range(8):
                m8 = smp.tile([128, 8], f32, name="m8", tag="m8")
                nc.vector.max(out=m8, in_=src[:, :W])
                nc.vector.match_replace(out=zap[:, :W], in_to_replace=m8,
                                        in_values=src[:, :W], imm_value=FMIN)
                src = zap
            # Masked positions carry smask(bf16)+score, not exactly FMIN; a
            # replaced masked slot would compare greater than FMIN and get
            # selected.  Clamping the zapped scores to -1e38 (far above any
            # masked value, far below any real score) kills those.
            nc.vector.tensor_scalar(zap[:, :W], zap[:, :W], -1.0e38, None,
                                    AL.max)

            m01 = mkp.tile([128, 2048], bf16, name="m01", tag="m01", bufs=1)
            nc.vector.tensor_tensor(m01[:, :W], scores[:, :W], zap[:, :W],
                                    AL.is_gt)
            wrt = mkp.tile([128, 2048], bf16, name="wrt", tag="wrt", bufs=1)
            nc.vector.tensor_tensor(wrt[:, :W], scores[:, :W], m01[:, :W],
                                    AL.mult)

            # ---- W^T and m01^T (both PE-transposed) ----
            wtsb = wtp.tile([128, 16, 128], bf16, name="wtsb", tag="wt")
            mtsb = wtp.tile([128, 16, 128], bf16, name="mtsb", tag="mt",
                            bufs=1)
            for src_t, dst in ((wrt, wtsb), (m01, mtsb)):
                for g in range((NT + 7) // 8):
                    n8 = min(8, NT - 8 * g)
                    pw = ps2.tile([128, 8, 128], bf16, name="pw", tag="pswt",
                                  bufs=1)
                    for i in range(n8):
                        st = 8 * g + i
                        nc.tensor.transpose(
                            pw[:, i, :], src_t[:, st * 128:(st + 1) * 128],
                            id_bf)
                    nc.any.tensor_copy(
                        dst[:, 8 * g:8 * g + n8, :].rearrange(
                            "p a c -> p (a c)"),
                        pw[:, :n8, :].rearrange("p a c -> p (a c)"))

            # ---- attention: PV accumulation + denominators fused ----
            # D[h, q] = sum_s exp(l)*m01^T accumulated with a ones-vector
            # matmul into a spare PSUM row of the pair's otp bank (cols
            # 128:256, partition 0 for the even head / 64 for the odd one).
            NG = NT // 4
            for he in range(0, H, 2):
                t6 = he // 2
                otp = psot.tile([128, 256], f32, name="otp", tag="psot")
                Dpair = psD.tile([128, 128], f32, name="Dpair", tag="Dp")
                for h in (he, he + 1):
                    hp = h % 2
                    pb = slice(64 * hp, 64 * hp + 64)
                    drow = Dpair[64 * hp:64 * hp + 1, :]
                    # keys x queries, [s, q] orientation -> P~T -> PV
                    for g in range(NG):
                        lt = ps2.tile([128, 512], f32, name="lt", tag="ps2")
                        ltv = lt.rearrange("p (a c) -> p a c", a=4)
                        for i in range(4):
                            st = 4 * g + i
                            nc.tensor.matmul(
                                ltv[:, i, :],
                                lhsT=KT[pb, t6, st * 128:(st + 1) * 128],
                                rhs=QT[pb, t6, qs])
                        et = etp.tile([128, 512], bf16, name="et", tag="et")
                        nc.scalar.activation(et, lt, AF.Exp, scale=0.125)
                        pt = ptp.tile([128, 512], bf16, name="pt", tag="pt")
                        nc.vector.tensor_tensor(
                            pt, et,
                            wtsb[:, 4 * g:4 * g + 4, :].rearrange(
                                "p a c -> p (a c)"), AL.mult)
                        ptv = pt.rearrange("p (a c) -> p a c", a=4)
                        jk = ptp.tile([128, 512], bf16, name="jk", tag="pt")
                        jkeng = nc.gpsimd if hp == 0 else nc.vector
                        jkeng.tensor_tensor(
                            jk, et,
                            mtsb[:, 4 * g:4 * g + 4, :].rearrange(
                                "p a c -> p (a c)"), AL.mult)
                        jkv = jk.rearrange("p (a c) -> p a c", a=4)
                        for i in range(4):
                            st = 4 * g + i
                            nc.tensor.matmul(
                                otp[pb, 0:128],
                                lhsT=Vt[:, st, h * 64:h * 64 + 64],
                                rhs=ptv[:, i, :], start=(st == 0),
                                stop=(st == NT - 1))
                            nc.tensor.matmul(
                                drow, lhsT=ones_bf, rhs=jkv[:, i, :],
                                start=(st == 0), stop=(st == NT - 1))
                if STAGE == "D":
                    dcp = smp.tile([65, 128], f32, name="dcp", tag="dcp", bufs=2)
                    nc.vector.tensor_copy(dcp[0:1, :], Dpair[0:1, :])
                    nc.vector.tensor_copy(dcp[64:65, :], Dpair[64:65, :])
                    nc.sync.dma_start(
                        out=out_d[j * 128 + 2 * t6:j * 128 + 2 * t6 + 1,
                                  0:128],
                        in_=dcp[0:1, :])
                    nc.sync.dma_start(
                        out=out_d[j * 128 + 2 * t6 + 64:
                                  j * 128 + 2 * t6 + 65, 0:128],
                        in_=dcp[64:65, :])
                # D for the pair, bounced via DRAM to broadcast each
                # head's row across the 64 partitions its PV rows occupy;
                # one full-tile reciprocal afterwards (cheap on 128 lanes)
                dD = smp.tile([65, 128], f32, name="dD", tag="ds")
                nc.scalar.copy(dD[0:1, :], Dpair[0:1, :])
                nc.scalar.copy(dD[64:65, :], Dpair[64:65, :])
                dscr0 = ddr.tile([1, 128], f32, name="dscr0", tag="dscr0")
                dscr1 = ddr.tile([1, 128], f32, name="dscr1", tag="dscr1")
                nc.sync.dma_start(out=dscr0, in_=dD[0:1, :])
                nc.sync.dma_start(out=dscr1, in_=dD[64:65, :])
                reps = rpp.tile([128, 128], f32, name="reps", tag="reps",
                                bufs=3)
                nc.sync.dma_start(out=reps[0:64, :],
                                  in_=dscr0.to_broadcast([64, 128]))
                nc.sync.dma_start(out=reps[64:128, :],
                                  in_=dscr1.to_broadcast([64, 128]))
                nc.vector.reciprocal(reps, reps)
                nc.vector.tensor_tensor(otsb[:, t6, qs], otp[:, 0:128],
                                        reps, AL.mult)

            # ---- output projection ----
            if STAGE == "D":
                continue
            for n0, nn in ((0, 512), (512, 256)):
                ops = ps2.tile([128, 512], f32, name="ops", tag="ps2")
                for kt in range(6):
                    nc.tensor.matmul(ops[:, :nn],
                                     lhsT=otsb[:, kt, qs],
                                     rhs=wout_sb[:, kt, n0:n0 + nn],
                                     start=(kt == 0), stop=(kt == 5))
                osb = smp.tile([128, 512], f32, name="osb", tag="osb", bufs=1)
                nc.any.tensor_copy(osb[:, :nn], ops[:, :nn])
                nc.sync.dma_start(out=out_d[qs, n0:n0 + nn], in_=osb[:, :nn])

    _px.close()


# ------------------------------------------------------------------
# host side
# ------------------------------------------------------------------
_CACHE = {}


def _install_ntff_hook():
    """The image lacks antenv.axon_hooks; rebuild it from trn_boot's
    ctypes NTFF profiler so run_bass_kernel_spmd(trace=True) works."""
    import sys
    import types
    if "antenv.axon_hooks" in sys.modules:
        return
    try:
        from trn_agent_boot.trn_boot import _ntff_profile_via_ctypes
        hook = _ntff_profile_via_ctypes("/opt/axon/libaxon_pjrt.so")
    except Exception:
        hook = None
    m = types.ModuleType("antenv.axon_hooks")
    m.get_axon_ntff_profile_hook = lambda: hook
    m.set_axon_ntff_profile_hook = lambda h: None
    sys.modules["antenv.axon_hooks"] = m


def make_inputs_for_core(c, x, wqkvT_bf, wiqT_s, wikT, woutT_bf):
    sm = slot_map(c)
    pbatch = sm[0][0]
    qbatch = sm[1][0]
    xT = [np.ascontiguousarray(x[b].T) for b in range(B)]
    xT_P = xT[pbatch]
    xT_Q = np.ascontiguousarray(xT[qbatch][:, :TQ])
    xTq = np.empty((D, 512), np.float32)
    smask = np.full((128, SMW), FMIN, np.float32)
    for j, (b, r, side) in enumerate(sm):
        xTq[:, j * 128:(j + 1) * 128] = xT[b][:, r * 128:(r + 1) * 128]
        Wj = SLOT_W[j]
        s = np.arange(Wj)[None, :]
        p = np.arange(128)[:, None]
        smask[:, SOFF[j]:SOFF[j] + Wj] = np.where(s <= 128 * r + p, 0.0, FMIN)
    return {
        "xT_P": xT_P, "xT_Q": xT_Q, "xTq": xTq,
        "wqkvT": wqkvT_bf, "wiqT": wiqT_s, "wikT": wikT, "woutT": woutT_bf,
        "smask": smask.astype(ml_dtypes.bfloat16),
    }


def kernel(x, wq_i, bq_i, wk_i, bk_i, w_head, w_qkv, b_qkv, w_out, b_out,
           trace=False):
    x = np.asarray(x, np.float32)
    for b_ in (bq_i, bk_i, b_qkv, b_out):
        assert np.abs(np.asarray(b_)).max() == 0.0, "nonzero bias unsupported"
    w_head = np.asarray(w_head, np.float32)
    signs = tuple(1 if s > 0 else -1 for s in w_head)

    import os
    key = (signs, os.environ.get("KSTAGE", "full"))
    if key not in _CACHE:
        _CACHE[key] = build_program(signs)
    nc = _CACHE[key]

    wqkvT_bf = np.ascontiguousarray(
        np.asarray(w_qkv, np.float32).T).astype(ml_dtypes.bfloat16)
    woutT_bf = np.ascontiguousarray(
        np.asarray(w_out, np.float32).T).astype(ml_dtypes.bfloat16)
    wiq = np.asarray(wq_i, np.float32).reshape(IH, ID, D) * \
        np.abs(w_head)[:, None, None]
    wiqT_s = np.ascontiguousarray(wiq.reshape(IH * ID, D).T)
    wikT = np.ascontiguousarray(np.asarray(wk_i, np.float32).T)

    in_maps = [make_inputs_for_core(c, x, wqkvT_bf, wiqT_s, wikT, woutT_bf)
               for c in range(NCORES)]
    kw = {}
    if trace:
        _install_ntff_hook()
        kw["trace_cores"] = list(range(NCORES))
    res = run_bass_kernel_spmd(nc, in_maps, core_ids=list(range(NCORES)),
                               trace=trace, **kw)

    out = np.empty((B, T, D), np.float32)
    for c in range(NCORES):
        oc = res.results[c]["out"]
        for j, (b, r, _s) in enumerate(slot_map(c)):
            out[b, r * 128:(r + 1) * 128, :] = oc[j * 128:(j + 1) * 128, :]
    kernel.last_result = res
    return out



# revision 25
# speedup vs baseline: 1.0308x; 1.0308x over previous
"""DeepSeek sparse attention (lightning indexer + top-64) on 8 trn2 cores.

Strategy (fully static SPMD program; per-core variation is data-only):
  - Each core owns 4 query tiles of 128 queries ("slots" with fixed key-widths
    512/1024/1536/2048).  Slot -> (batch, qtile) assignment is done on the
    host; each core uploads xT for the two batches it touches ("P" side full
    2048 cols, "Q" side first 1536 cols), the 512 query columns, and an
    additive causal/pad mask per slot.
  - Indexer path (projections + qI.kI scores) runs in fp32 (float32r matmuls)
    so the top-64 selection matches the fp32 reference bit-nearly-exactly.
  - Top-64 per query via 8 rounds of max8 + match_replace; selection becomes
    a 0/1 mask (score > zapped) and routing weights W = score * mask.
  - Attention avoids any gather: logits are computed densely in BOTH
    orientations.  [key, query] orientation: P~T = exp(L/8) * W^T feeds PV
    matmuls directly (masked positions have W == 0).  [query, key]
    orientation feeds exp + masked row-sum (tensor_tensor_reduce) to get the
    softmax denominator.  1/denominator is applied on the tiny PV output.
  - Output projection consumes the feature-major attention output, yielding
    token-major [512, 768] per core, DMA'd straight from PSUM.
"""

import numpy as np
import ml_dtypes

import concourse.bass as bass
import concourse.bacc as bacc
import concourse.mybir as mybir
import concourse.tile as tile
from concourse.bass_utils import run_bass_kernel_spmd

f32 = mybir.dt.float32
f32r = mybir.dt.float32r
bf16 = mybir.dt.bfloat16
AL = mybir.AluOpType
AF = mybir.ActivationFunctionType
AX = mybir.AxisListType

FMIN = -3.0e38
B, T, D = 2, 2048, 768
H, DH, IH, ID, TOPK = 12, 64, 4, 64, 64
TP, TQ = 2048, 1536          # key extents kept for the two batch "sides"
SLOT_W = (512, 1024, 1536, 2048)
SOFF = (0, 512, 1536, 3072)  # smask column offset per slot
SMW = 5120
NCORES = 8


def slot_map(c):
    """slot j -> (batch, qtile_index, side)   side 0 = "P", 1 = "Q"."""
    d, p = c // 2, c % 2
    q = 1 - p
    return [(p, d, 0), (q, 7 - d, 1), (q, 8 + d, 1), (p, 15 - d, 0)]


def _r(ap):
    """[768, C] dram/sbuf view -> [128, 6, C]."""
    return ap.rearrange("(a p) c -> p a c", p=128)


def build_program(signs):
    import os
    _ = os.environ.get("KSTAGE", "full")
    nc = bacc.Bacc("TRN2", target_bir_lowering=False, debug=False,
                   num_devices=NCORES)

    xT_P = nc.dram_tensor("xT_P", [D, TP], f32, kind="ExternalInput").ap()
    xT_Q = nc.dram_tensor("xT_Q", [D, TQ], f32, kind="ExternalInput").ap()
    xTq = nc.dram_tensor("xTq", [D, 512], f32, kind="ExternalInput").ap()
    wqkvT = nc.dram_tensor("wqkvT", [D, 3 * D], bf16, kind="ExternalInput").ap()
    wiqT = nc.dram_tensor("wiqT", [D, IH * ID], f32, kind="ExternalInput").ap()
    wikT = nc.dram_tensor("wikT", [D, IH * ID], f32, kind="ExternalInput").ap()
    woutT = nc.dram_tensor("woutT", [D, D], bf16, kind="ExternalInput").ap()
    smask = nc.dram_tensor("smask", [128, SMW], bf16, kind="ExternalInput").ap()
    out_d = nc.dram_tensor("out", [512, D], f32, kind="ExternalOutput").ap()

    with tile.TileContext(nc) as tc:
        _body(tc, xT_P, xT_Q, xTq, wqkvT, wiqT, wikT, woutT, smask, out_d,
              signs)
    nc.compile()
    return nc


def _body(tc, xT_P, xT_Q, xTq, wqkvT, wiqT, wikT, woutT, smask, out_d, signs):
    nc = tc.nc
    import os
    STAGE = os.environ.get("KSTAGE", "full")

    # ---------------- persistent tensors ----------------
    from contextlib import ExitStack
    _px = ExitStack()
    pp = _px.enter_context(tc.tile_pool(name="persist", bufs=1))
    KT_P = pp.tile([128, 6, TP], bf16, name="KT_P")
    KT_Q = pp.tile([128, 6, TQ], bf16, name="KT_Q")
    V_P = pp.tile([128, TP // 128, D], bf16, name="V_P")
    V_Q = pp.tile([128, TQ // 128, D], bf16, name="V_Q")
    kIT_P = pp.tile([128, 2, TP], f32r, name="kIT_P")
    kIT_Q = pp.tile([128, 2, TQ], f32r, name="kIT_Q")
    QT = pp.tile([128, 6, 512], bf16, name="QT")
    qIT = pp.tile([128, 2, 512], f32r, name="qIT")
    otsb = pp.tile([128, 6, 512], bf16, name="otsb")
    id_bf = pp.tile([128, 128], bf16, name="id_bf")
    ones_bf = pp.tile([128, 1], bf16, name="ones_bf")

    # identity matrix for PE transposes
    nc.vector.memset(id_bf, 1.0)
    nc.gpsimd.affine_select(id_bf, id_bf, pattern=[[-1, 128]],
                            compare_op=AL.is_equal, fill=0.0, base=0,
                            channel_multiplier=1)
    nc.vector.memset(ones_bf, 1.0)

    # ---------------- stage 1: projections ----------------
    CH = 256
    with tc.tile_pool(name="w1", bufs=1) as wp, \
         tc.tile_pool(name="ps1", bufs=3, space="PSUM") as ps1, \
         tc.tile_pool(name="psv", bufs=2, space="PSUM") as psv:
        wqkv_sb = wp.tile([128, 6, 3 * D], bf16)
        wiq_sb = wp.tile([128, 6, IH * ID], f32)
        wik_sb = wp.tile([128, 6, IH * ID], f32)
        nc.sync.dma_start(out=wqkv_sb, in_=_r(wqkvT))
        nc.sync.dma_start(out=wiq_sb, in_=_r(wiqT))
        nc.sync.dma_start(out=wik_sb, in_=_r(wikT))
        # fp32r operands must come from an op that rounds to fp32r
        wiq_r = wp.tile([128, 6, IH * ID], f32r)
        wik_r = wp.tile([128, 6, IH * ID], f32r)
        nc.any.tensor_copy(wiq_r.rearrange("p a c -> p (a c)"),
                           wiq_sb.rearrange("p a c -> p (a c)"))
        nc.any.tensor_copy(wik_r.rearrange("p a c -> p (a c)"),
                           wik_sb.rearrange("p a c -> p (a c)"))

        # queries first (they are needed by every slot)
        with tc.tile_pool(name="xq", bufs=1) as xqp:
            xqv = _r(xTq)
            for q0 in range(0, 512, 256):
                xq = xqp.tile([128, 6, 256], f32, name="xq", tag="xq")
                nc.sync.dma_start(out=xq, in_=xqv[:, :, q0:q0 + 256])
                xqb = xqp.tile([128, 6, 256], bf16, name="xqb", tag="xqb")
                nc.any.tensor_copy(xqb.rearrange("p a c -> p (a c)"),
                                   xq.rearrange("p a c -> p (a c)"))
                xqr = xqp.tile([128, 6, 256], f32r, name="xqr", tag="xqr")
                nc.gpsimd.tensor_copy(xqr.rearrange("p a c -> p (a c)"),
                                      xq.rearrange("p a c -> p (a c)"))
                for mt in range(2):
                    ps = ps1.tile([128, 256], f32, name="ps_qi", tag="ps1")
                    for kt in range(6):
                        nc.tensor.matmul(
                            ps, lhsT=wiq_r[:, kt, mt * 128:(mt + 1) * 128],
                            rhs=xqr[:, kt, :], start=(kt == 0), stop=(kt == 5))
                    nc.any.tensor_copy(qIT[:, mt, q0:q0 + 256], ps)
                for mt in range(6):
                    ps = ps1.tile([128, 256], f32, name="ps_qt", tag="ps1")
                    for kt in range(6):
                        nc.tensor.matmul(
                            ps, lhsT=wqkv_sb[:, kt, mt * 128:(mt + 1) * 128],
                            rhs=xqb[:, kt, :], start=(kt == 0), stop=(kt == 5))
                    nc.any.tensor_copy(QT[:, mt, q0:q0 + 256], ps)

        with tc.tile_pool(name="xc", bufs=2) as xcp, \
             tc.tile_pool(name="xcb", bufs=2) as xbp:
            for side, (xT, S) in enumerate(((xT_P, TP), (xT_Q, TQ))):
                xv = _r(xT)
                KT = KT_P if side == 0 else KT_Q
                Vt = V_P if side == 0 else V_Q
                kIT = kIT_P if side == 0 else kIT_Q
                for c0 in range(0, S, CH):
                    xc = xcp.tile([128, 6, CH], f32, name="xc", tag="xc")
                    nc.sync.dma_start(out=xc, in_=xv[:, :, c0:c0 + CH])
                    xcb = xbp.tile([128, 6, CH], bf16, name="xcb", tag="xcb")
                    nc.any.tensor_copy(xcb.rearrange("p a c -> p (a c)"),
                                       xc.rearrange("p a c -> p (a c)"))
                    xcr = xbp.tile([128, 6, CH], f32r, name="xcr", tag="xcr", bufs=1)
                    nc.gpsimd.tensor_copy(xcr.rearrange("p a c -> p (a c)"),
                                          xc.rearrange("p a c -> p (a c)"))
                    # indexer keys (fp32r, exact)
                    for mt in range(2):
                        ps = ps1.tile([128, CH], f32, name="ps_ki", tag="ps1")
                        for kt in range(6):
                            nc.tensor.matmul(
                                ps, lhsT=wik_r[:, kt, mt * 128:(mt + 1) * 128],
                                rhs=xcr[:, kt, :],
                                start=(kt == 0), stop=(kt == 5))
                        nc.any.tensor_copy(kIT[:, mt, c0:c0 + CH], ps)
                    # attention keys KT (bf16)
                    for mt in range(6):
                        ps = ps1.tile([128, CH], f32, name="ps_kt", tag="ps1")
                        for kt in range(6):
                            nc.tensor.matmul(
                                ps,
                                lhsT=wqkv_sb[:, kt,
                                             D + mt * 128:D + (mt + 1) * 128],
                                rhs=xcb[:, kt, :], start=(kt == 0),
                                stop=(kt == 5))
                        nc.any.tensor_copy(KT[:, mt, c0:c0 + CH], ps)
                    # values V (token-major, bf16)
                    for st in range(CH // 128):
                        ps = psv.tile([128, D], f32, name="ps_v", tag="psv")
                        for n0, nn in ((0, 512), (512, 256)):
                            for kt in range(6):
                                nc.tensor.matmul(
                                    ps[:, n0:n0 + nn],
                                    lhsT=xcb[:, kt, st * 128:(st + 1) * 128],
                                    rhs=wqkv_sb[:, kt,
                                                2 * D + n0:2 * D + n0 + nn],
                                    start=(kt == 0), stop=(kt == 5))
                        nc.any.tensor_copy(Vt[:, c0 // 128 + st, :], ps)

    # ---------------- stage 2/3: per query tile ----------------
    with tc.tile_pool(name="w2", bufs=1) as wp2, \
         tc.tile_pool(name="sc", bufs=2) as scp, \
         tc.tile_pool(name="zap", bufs=1) as zpp, \
         tc.tile_pool(name="msk", bufs=2) as mkp, \
         tc.tile_pool(name="wts", bufs=2) as wtp, \
         tc.tile_pool(name="u", bufs=2) as upl, \
         tc.tile_pool(name="et", bufs=2 if STAGE == "D" else 3) as etp, \
         tc.tile_pool(name="pt", bufs=3) as ptp, \
         tc.tile_pool(name="sm", bufs=4) as smp, \
         tc.tile_pool(name="rp", bufs=1) as rpp, \
         tc.tile_pool(name="ps2", bufs=2, space="PSUM") as ps2, \
         tc.tile_pool(name="psot", bufs=3, space="PSUM") as psot, \
         tc.tile_pool(name="psD", bufs=2, space="PSUM") as psD, \
         tc.tile_pool(name="ddr", bufs=2, space="DRAM") as ddr:
        wout_sb = wp2.tile([128, 6, D], bf16)
        nc.sync.dma_start(out=wout_sb, in_=_r(woutT))
        smask_sb = wp2.tile([128, SMW], bf16)
        nc.sync.dma_start(out=smask_sb, in_=smask)

        for j in (0, 3, 2, 1):
            W = SLOT_W[j]
            side = (0, 1, 1, 0)[j]
            KT = KT_P if side == 0 else KT_Q
            Vt = V_P if side == 0 else V_Q
            kIT = kIT_P if side == 0 else kIT_Q
            NT = W // 128
            NCH = W // 512
            qs = slice(j * 128, (j + 1) * 128)

            # ---- indexer scores ----
            scores = scp.tile([128, 2048], f32, name="scores", tag="scores")
            for ch in range(NCH):
                cs = slice(ch * 512, (ch + 1) * 512)
                us = []
                for h4 in range(IH):
                    hp, t2 = h4 % 2, h4 // 2
                    ps = ps2.tile([128, 512], f32, name="ps_ix", tag="ps2")
                    nc.tensor.matmul(
                        ps, lhsT=qIT[64 * hp:64 * hp + 64, t2, qs],
                        rhs=kIT[64 * hp:64 * hp + 64, t2, cs])
                    uh = upl.tile([128, 512], f32, name="u", tag="u")
                    nc.scalar.activation(uh, ps, AF.Relu)
                    us.append(uh)
                # signed head-sum + causal/pad mask, all on gpsimd:
                # sc = smask +- u0 +- u1 +- u2 +- u3  (|w_h| folded into qIT)
                smk = smask_sb[:, SOFF[j] + ch * 512:SOFF[j] + (ch + 1) * 512]
                sc = scores[:, cs]
                if signs[0] > 0:
                    nc.gpsimd.tensor_tensor(sc, us[0], smk, AL.add)
                else:
                    nc.gpsimd.tensor_tensor(sc, smk, us[0], AL.subtract)
                for h4 in range(1, IH):
                    nc.gpsimd.tensor_tensor(
                        sc, sc, us[h4],
                        AL.add if signs[h4] > 0 else AL.subtract)

            # ---- top-64 threshold via 8x (max8 + match_replace) ----
            zap = zpp.tile([128, 2048], f32, name="zap", tag="zap")
            src = scores
            for r in range(8):
                m8 = smp.tile([128, 8], f32, name="m8", tag="m8")
                nc.vector.max(out=m8, in_=src[:, :W])
                nc.vector.match_replace(out=zap[:, :W], in_to_replace=m8,
                                        in_values=src[:, :W], imm_value=FMIN)
                src = zap
            # Masked positions carry smask(bf16)+score, not exactly FMIN; a
            # replaced masked slot would compare greater than FMIN and get
            # selected.  Clamping the zapped scores to -1e38 (far above any
            # masked value, far below any real score) kills those.
            nc.vector.tensor_scalar(zap[:, :W], zap[:, :W], -1.0e38, None,
                                    AL.max)

            m01 = mkp.tile([128, 2048], bf16, name="m01", tag="m01", bufs=1)
            nc.vector.tensor_tensor(m01[:, :W], scores[:, :W], zap[:, :W],
                                    AL.is_gt)
            wrt = mkp.tile([128, 2048], bf16, name="wrt", tag="wrt", bufs=1)
            nc.vector.tensor_tensor(wrt[:, :W], scores[:, :W], m01[:, :W],
                                    AL.mult)

            # ---- W^T and m01^T (both PE-transposed) ----
            wtsb = wtp.tile([128, 16, 128], bf16, name="wtsb", tag="wt")
            mtsb = wtp.tile([128, 16, 128], bf16, name="mtsb", tag="mt",
                            bufs=1)
            for src_t, dst in ((wrt, wtsb), (m01, mtsb)):
                for g in range((NT + 7) // 8):
                    n8 = min(8, NT - 8 * g)
                    pw = ps2.tile([128, 8, 128], bf16, name="pw", tag="pswt",
                                  bufs=1)
                    for i in range(n8):
                        st = 8 * g + i
                        nc.tensor.transpose(
                            pw[:, i, :], src_t[:, st * 128:(st + 1) * 128],
                            id_bf)
                    nc.any.tensor_copy(
                        dst[:, 8 * g:8 * g + n8, :].rearrange(
                            "p a c -> p (a c)"),
                        pw[:, :n8, :].rearrange("p a c -> p (a c)"))

            # ---- attention: PV accumulation + denominators fused ----
            # D[h, q] = sum_s exp(l)*m01^T accumulated with a ones-vector
            # matmul into a spare PSUM row of the pair's otp bank (cols
            # 128:256, partition 0 for the even head / 64 for the odd one).
            NG = NT // 4
            for he in range(0, H, 2):
                t6 = he // 2
                otp = psot.tile([128, 256], f32, name="otp", tag="psot")
                Dpair = psD.tile([128, 128], f32, name="Dpair", tag="Dp")
                for h in (he, he + 1):
                    hp = h % 2
                    pb = slice(64 * hp, 64 * hp + 64)
                    drow = Dpair[64 * hp:64 * hp + 1, :]
                    # keys x queries, [s, q] orientation -> P~T -> PV
                    for g in range(NG):
                        lt = ps2.tile([128, 512], f32, name="lt", tag="ps2")
                        ltv = lt.rearrange("p (a c) -> p a c", a=4)
                        for i in range(4):
                            st = 4 * g + i
                            nc.tensor.matmul(
                                ltv[:, i, :],
                                lhsT=KT[pb, t6, st * 128:(st + 1) * 128],
                                rhs=QT[pb, t6, qs])
                        et = etp.tile([128, 512], bf16, name="et", tag="et")
                        nc.scalar.activation(et, lt, AF.Exp, scale=0.125)
                        pt = ptp.tile([128, 512], bf16, name="pt", tag="pt")
                        nc.vector.tensor_tensor(
                            pt, et,
                            wtsb[:, 4 * g:4 * g + 4, :].rearrange(
                                "p a c -> p (a c)"), AL.mult)
                        ptv = pt.rearrange("p (a c) -> p a c", a=4)
                        jk = ptp.tile([128, 512], bf16, name="jk", tag="pt")
                        nc.vector.tensor_tensor(
                            jk, et,
                            mtsb[:, 4 * g:4 * g + 4, :].rearrange(
                                "p a c -> p (a c)"), AL.mult)
                        jkv = jk.rearrange("p (a c) -> p a c", a=4)
                        for i in range(4):
                            st = 4 * g + i
                            nc.tensor.matmul(
                                otp[pb, 0:128],
                                lhsT=Vt[:, st, h * 64:h * 64 + 64],
                                rhs=ptv[:, i, :], start=(st == 0),
                                stop=(st == NT - 1))
                            nc.tensor.matmul(
                                drow, lhsT=ones_bf, rhs=jkv[:, i, :],
                                start=(st == 0), stop=(st == NT - 1))
                if STAGE == "D":
                    dcp = smp.tile([65, 128], f32, name="dcp", tag="dcp", bufs=2)
                    nc.vector.tensor_copy(dcp[0:1, :], Dpair[0:1, :])
                    nc.vector.tensor_copy(dcp[64:65, :], Dpair[64:65, :])
                    nc.sync.dma_start(
                        out=out_d[j * 128 + 2 * t6:j * 128 + 2 * t6 + 1,
                                  0:128],
                        in_=dcp[0:1, :])
                    nc.sync.dma_start(
                        out=out_d[j * 128 + 2 * t6 + 64:
                                  j * 128 + 2 * t6 + 65, 0:128],
                        in_=dcp[64:65, :])
                # D for the pair, bounced via DRAM to broadcast each
                # head's row across the 64 partitions its PV rows occupy;
                # one full-tile reciprocal afterwards (cheap on 128 lanes)
                dD = smp.tile([65, 128], f32, name="dD", tag="ds")
                nc.scalar.copy(dD[0:1, :], Dpair[0:1, :])
                nc.scalar.copy(dD[64:65, :], Dpair[64:65, :])
                dscr0 = ddr.tile([1, 128], f32, name="dscr0", tag="dscr0")
                dscr1 = ddr.tile([1, 128], f32, name="dscr1", tag="dscr1")
                nc.sync.dma_start(out=dscr0, in_=dD[0:1, :])
                nc.sync.dma_start(out=dscr1, in_=dD[64:65, :])
                reps = rpp.tile([128, 128], f32, name="reps", tag="reps",
                                bufs=3)
                nc.sync.dma_start(out=reps[0:64, :],
                                  in_=dscr0.to_broadcast([64, 128]))
                nc.sync.dma_start(out=reps[64:128, :],
                                  in_=dscr1.to_broadcast([64, 128]))
                nc.vector.reciprocal(reps, reps)
                nc.vector.tensor_tensor(otsb[:, t6, qs], otp[:, 0:128],
                                        reps, AL.mult)

            # ---- output projection ----
            if STAGE == "D":
                continue
            for n0, nn in ((0, 512), (512, 256)):
                ops = ps2.tile([128, 512], f32, name="ops", tag="ps2")
                for kt in range(6):
                    nc.tensor.matmul(ops[:, :nn],
                                     lhsT=otsb[:, kt, qs],
                                     rhs=wout_sb[:, kt, n0:n0 + nn],
                                     start=(kt == 0), stop=(kt == 5))
                osb = smp.tile([128, 512], f32, name="osb", tag="osb", bufs=1)
                nc.any.tensor_copy(osb[:, :nn], ops[:, :nn])
                nc.sync.dma_start(out=out_d[qs, n0:n0 + nn], in_=osb[:, :nn])

    _px.close()


# ------------------------------------------------------------------
# host side
# ------------------------------------------------------------------
_CACHE = {}


def _install_ntff_hook():
    """The image lacks antenv.axon_hooks; rebuild it from trn_boot's
    ctypes NTFF profiler so run_bass_kernel_spmd(trace=True) works."""
    import sys
    import types
    if "antenv.axon_hooks" in sys.modules:
        return
    try:
        from trn_agent_boot.trn_boot import _ntff_profile_via_ctypes
        hook = _ntff_profile_via_ctypes("/opt/axon/libaxon_pjrt.so")
    except Exception:
        hook = None
    m = types.ModuleType("antenv.axon_hooks")
    m.get_axon_ntff_profile_hook = lambda: hook
    m.set_axon_ntff_profile_hook = lambda h: None
    sys.modules["antenv.axon_hooks"] = m


def make_inputs_for_core(c, x, wqkvT_bf, wiqT_s, wikT, woutT_bf):
    sm = slot_map(c)
    pbatch = sm[0][0]
    qbatch = sm[1][0]
    xT = [np.ascontiguousarray(x[b].T) for b in range(B)]
    xT_P = xT[pbatch]
    xT_Q = np.ascontiguousarray(xT[qbatch][:, :TQ])
    xTq = np.empty((D, 512), np.float32)
    smask = np.full((128, SMW), FMIN, np.float32)
    for j, (b, r, side) in enumerate(sm):
        xTq[:, j * 128:(j + 1) * 128] = xT[b][:, r * 128:(r + 1) * 128]
        Wj = SLOT_W[j]
        s = np.arange(Wj)[None, :]
        p = np.arange(128)[:, None]
        smask[:, SOFF[j]:SOFF[j] + Wj] = np.where(s <= 128 * r + p, 0.0, FMIN)
    return {
        "xT_P": xT_P, "xT_Q": xT_Q, "xTq": xTq,
        "wqkvT": wqkvT_bf, "wiqT": wiqT_s, "wikT": wikT, "woutT": woutT_bf,
        "smask": smask.astype(ml_dtypes.bfloat16),
    }


def kernel(x, wq_i, bq_i, wk_i, bk_i, w_head, w_qkv, b_qkv, w_out, b_out,
           trace=False):
    x = np.asarray(x, np.float32)
    for b_ in (bq_i, bk_i, b_qkv, b_out):
        assert np.abs(np.asarray(b_)).max() == 0.0, "nonzero bias unsupported"
    w_head = np.asarray(w_head, np.float32)
    signs = tuple(1 if s > 0 else -1 for s in w_head)

    import os
    key = (signs, os.environ.get("KSTAGE", "full"))
    if key not in _CACHE:
        _CACHE[key] = build_program(signs)
    nc = _CACHE[key]

    wqkvT_bf = np.ascontiguousarray(
        np.asarray(w_qkv, np.float32).T).astype(ml_dtypes.bfloat16)
    woutT_bf = np.ascontiguousarray(
        np.asarray(w_out, np.float32).T).astype(ml_dtypes.bfloat16)
    wiq = np.asarray(wq_i, np.float32).reshape(IH, ID, D) * \
        np.abs(w_head)[:, None, None]
    wiqT_s = np.ascontiguousarray(wiq.reshape(IH * ID, D).T)
    wikT = np.ascontiguousarray(np.asarray(wk_i, np.float32).T)

    in_maps = [make_inputs_for_core(c, x, wqkvT_bf, wiqT_s, wikT, woutT_bf)
               for c in range(NCORES)]
    kw = {}
    if trace:
        _install_ntff_hook()
        kw["trace_cores"] = list(range(NCORES))
    res = run_bass_kernel_spmd(nc, in_maps, core_ids=list(range(NCORES)),
                               trace=trace, **kw)

    out = np.empty((B, T, D), np.float32)
    for c in range(NCORES):
        oc = res.results[c]["out"]
        for j, (b, r, _s) in enumerate(slot_map(c)):
            out[b, r * 128:(r + 1) * 128, :] = oc[j * 128:(j + 1) * 128, :]
    kernel.last_result = res
    return out



# revision 26
# speedup vs baseline: 1.0423x; 1.0112x over previous
"""DeepSeek sparse attention (lightning indexer + top-64) on 8 trn2 cores.

Strategy (fully static SPMD program; per-core variation is data-only):
  - Each core owns 4 query tiles of 128 queries ("slots" with fixed key-widths
    512/1024/1536/2048).  Slot -> (batch, qtile) assignment is done on the
    host; each core uploads xT for the two batches it touches ("P" side full
    2048 cols, "Q" side first 1536 cols), the 512 query columns, and an
    additive causal/pad mask per slot.
  - Indexer path (projections + qI.kI scores) runs in fp32 (float32r matmuls)
    so the top-64 selection matches the fp32 reference bit-nearly-exactly.
  - Top-64 per query via 8 rounds of max8 + match_replace; selection becomes
    a 0/1 mask (score > zapped) and routing weights W = score * mask.
  - Attention avoids any gather: logits are computed densely in BOTH
    orientations.  [key, query] orientation: P~T = exp(L/8) * W^T feeds PV
    matmuls directly (masked positions have W == 0).  [query, key]
    orientation feeds exp + masked row-sum (tensor_tensor_reduce) to get the
    softmax denominator.  1/denominator is applied on the tiny PV output.
  - Output projection consumes the feature-major attention output, yielding
    token-major [512, 768] per core, DMA'd straight from PSUM.
"""

import numpy as np
import ml_dtypes

import concourse.bass as bass
import concourse.bacc as bacc
import concourse.mybir as mybir
import concourse.tile as tile
from concourse.bass_utils import run_bass_kernel_spmd

f32 = mybir.dt.float32
f32r = mybir.dt.float32r
bf16 = mybir.dt.bfloat16
AL = mybir.AluOpType
AF = mybir.ActivationFunctionType
AX = mybir.AxisListType

FMIN = -3.0e38
B, T, D = 2, 2048, 768
H, DH, IH, ID, TOPK = 12, 64, 4, 64, 64
TP, TQ = 2048, 1536          # key extents kept for the two batch "sides"
SLOT_W = (512, 1024, 1536, 2048)
SOFF = (0, 512, 1536, 3072)  # smask column offset per slot
SMW = 5120
NCORES = 8


def slot_map(c):
    """slot j -> (batch, qtile_index, side)   side 0 = "P", 1 = "Q"."""
    d, p = c // 2, c % 2
    q = 1 - p
    return [(p, d, 0), (q, 7 - d, 1), (q, 8 + d, 1), (p, 15 - d, 0)]


def _r(ap):
    """[768, C] dram/sbuf view -> [128, 6, C]."""
    return ap.rearrange("(a p) c -> p a c", p=128)


def build_program(signs):
    import os
    _ = os.environ.get("KSTAGE", "full")
    nc = bacc.Bacc("TRN2", target_bir_lowering=False, debug=False,
                   num_devices=NCORES)

    xT_P = nc.dram_tensor("xT_P", [D, TP], f32, kind="ExternalInput").ap()
    xT_Q = nc.dram_tensor("xT_Q", [D, TQ], f32, kind="ExternalInput").ap()
    xTq = nc.dram_tensor("xTq", [D, 512], f32, kind="ExternalInput").ap()
    wqkvT = nc.dram_tensor("wqkvT", [D, 3 * D], bf16, kind="ExternalInput").ap()
    wiqT = nc.dram_tensor("wiqT", [D, IH * ID], f32, kind="ExternalInput").ap()
    wikT = nc.dram_tensor("wikT", [D, IH * ID], f32, kind="ExternalInput").ap()
    woutT = nc.dram_tensor("woutT", [D, D], bf16, kind="ExternalInput").ap()
    smask = nc.dram_tensor("smask", [128, SMW], bf16, kind="ExternalInput").ap()
    out_d = nc.dram_tensor("out", [512, D], f32, kind="ExternalOutput").ap()

    with tile.TileContext(nc) as tc:
        _body(tc, xT_P, xT_Q, xTq, wqkvT, wiqT, wikT, woutT, smask, out_d,
              signs)
    nc.compile()
    return nc


def _body(tc, xT_P, xT_Q, xTq, wqkvT, wiqT, wikT, woutT, smask, out_d, signs):
    nc = tc.nc
    import os
    STAGE = os.environ.get("KSTAGE", "full")

    # ---------------- persistent tensors ----------------
    from contextlib import ExitStack
    _px = ExitStack()
    pp = _px.enter_context(tc.tile_pool(name="persist", bufs=1))
    KT_P = pp.tile([128, 6, TP], bf16, name="KT_P")
    KT_Q = pp.tile([128, 6, TQ], bf16, name="KT_Q")
    V_P = pp.tile([128, TP // 128, D], bf16, name="V_P")
    V_Q = pp.tile([128, TQ // 128, D], bf16, name="V_Q")
    kIT_P = pp.tile([128, 2, TP], f32r, name="kIT_P")
    kIT_Q = pp.tile([128, 2, TQ], f32r, name="kIT_Q")
    QT = pp.tile([128, 6, 512], bf16, name="QT")
    qIT = pp.tile([128, 2, 512], f32r, name="qIT")
    otsb = pp.tile([128, 6, 512], bf16, name="otsb")
    id_bf = pp.tile([128, 128], bf16, name="id_bf")
    ones_bf = pp.tile([128, 1], bf16, name="ones_bf")

    # identity matrix for PE transposes
    nc.vector.memset(id_bf, 1.0)
    nc.gpsimd.affine_select(id_bf, id_bf, pattern=[[-1, 128]],
                            compare_op=AL.is_equal, fill=0.0, base=0,
                            channel_multiplier=1)
    nc.vector.memset(ones_bf, 1.0)

    # ---------------- stage 1: projections ----------------
    CH = 256
    with tc.tile_pool(name="w1", bufs=1) as wp, \
         tc.tile_pool(name="ps1", bufs=3, space="PSUM") as ps1, \
         tc.tile_pool(name="psv", bufs=2, space="PSUM") as psv:
        wqkv_sb = wp.tile([128, 6, 3 * D], bf16)
        wiq_sb = wp.tile([128, 6, IH * ID], f32)
        wik_sb = wp.tile([128, 6, IH * ID], f32)
        nc.sync.dma_start(out=wqkv_sb, in_=_r(wqkvT))
        nc.sync.dma_start(out=wiq_sb, in_=_r(wiqT))
        nc.sync.dma_start(out=wik_sb, in_=_r(wikT))
        # fp32r operands must come from an op that rounds to fp32r
        wiq_r = wp.tile([128, 6, IH * ID], f32r)
        wik_r = wp.tile([128, 6, IH * ID], f32r)
        nc.any.tensor_copy(wiq_r.rearrange("p a c -> p (a c)"),
                           wiq_sb.rearrange("p a c -> p (a c)"))
        nc.any.tensor_copy(wik_r.rearrange("p a c -> p (a c)"),
                           wik_sb.rearrange("p a c -> p (a c)"))

        # queries first (they are needed by every slot)
        with tc.tile_pool(name="xq", bufs=1) as xqp:
            xqv = _r(xTq)
            for q0 in range(0, 512, 256):
                xq = xqp.tile([128, 6, 256], f32, name="xq", tag="xq")
                nc.sync.dma_start(out=xq, in_=xqv[:, :, q0:q0 + 256])
                xqb = xqp.tile([128, 6, 256], bf16, name="xqb", tag="xqb")
                nc.any.tensor_copy(xqb.rearrange("p a c -> p (a c)"),
                                   xq.rearrange("p a c -> p (a c)"))
                xqr = xqp.tile([128, 6, 256], f32r, name="xqr", tag="xqr")
                nc.any.tensor_copy(xqr.rearrange("p a c -> p (a c)"),
                                   xq.rearrange("p a c -> p (a c)"))
                for mt in range(2):
                    ps = ps1.tile([128, 256], f32, name="ps_qi", tag="ps1")
                    for kt in range(6):
                        nc.tensor.matmul(
                            ps, lhsT=wiq_r[:, kt, mt * 128:(mt + 1) * 128],
                            rhs=xqr[:, kt, :], start=(kt == 0), stop=(kt == 5))
                    nc.any.tensor_copy(qIT[:, mt, q0:q0 + 256], ps)
                for mt in range(6):
                    ps = ps1.tile([128, 256], f32, name="ps_qt", tag="ps1")
                    for kt in range(6):
                        nc.tensor.matmul(
                            ps, lhsT=wqkv_sb[:, kt, mt * 128:(mt + 1) * 128],
                            rhs=xqb[:, kt, :], start=(kt == 0), stop=(kt == 5))
                    nc.any.tensor_copy(QT[:, mt, q0:q0 + 256], ps)

        with tc.tile_pool(name="xc", bufs=2) as xcp, \
             tc.tile_pool(name="xcb", bufs=2) as xbp:
            for side, (xT, S) in enumerate(((xT_P, TP), (xT_Q, TQ))):
                xv = _r(xT)
                KT = KT_P if side == 0 else KT_Q
                Vt = V_P if side == 0 else V_Q
                kIT = kIT_P if side == 0 else kIT_Q
                for c0 in range(0, S, CH):
                    xc = xcp.tile([128, 6, CH], f32, name="xc", tag="xc")
                    nc.sync.dma_start(out=xc, in_=xv[:, :, c0:c0 + CH])
                    xcb = xbp.tile([128, 6, CH], bf16, name="xcb", tag="xcb")
                    nc.any.tensor_copy(xcb.rearrange("p a c -> p (a c)"),
                                       xc.rearrange("p a c -> p (a c)"))
                    xcr = xbp.tile([128, 6, CH], f32r, name="xcr", tag="xcr", bufs=1)
                    nc.any.tensor_copy(xcr.rearrange("p a c -> p (a c)"),
                                       xc.rearrange("p a c -> p (a c)"))
                    # indexer keys (fp32r, exact)
                    for mt in range(2):
                        ps = ps1.tile([128, CH], f32, name="ps_ki", tag="ps1")
                        for kt in range(6):
                            nc.tensor.matmul(
                                ps, lhsT=wik_r[:, kt, mt * 128:(mt + 1) * 128],
                                rhs=xcr[:, kt, :],
                                start=(kt == 0), stop=(kt == 5))
                        nc.any.tensor_copy(kIT[:, mt, c0:c0 + CH], ps)
                    # attention keys KT (bf16)
                    for mt in range(6):
                        ps = ps1.tile([128, CH], f32, name="ps_kt", tag="ps1")
                        for kt in range(6):
                            nc.tensor.matmul(
                                ps,
                                lhsT=wqkv_sb[:, kt,
                                             D + mt * 128:D + (mt + 1) * 128],
                                rhs=xcb[:, kt, :], start=(kt == 0),
                                stop=(kt == 5))
                        nc.any.tensor_copy(KT[:, mt, c0:c0 + CH], ps)
                    # values V (token-major, bf16)
                    for st in range(CH // 128):
                        ps = psv.tile([128, D], f32, name="ps_v", tag="psv")
                        for n0, nn in ((0, 512), (512, 256)):
                            for kt in range(6):
                                nc.tensor.matmul(
                                    ps[:, n0:n0 + nn],
                                    lhsT=xcb[:, kt, st * 128:(st + 1) * 128],
                                    rhs=wqkv_sb[:, kt,
                                                2 * D + n0:2 * D + n0 + nn],
                                    start=(kt == 0), stop=(kt == 5))
                        nc.any.tensor_copy(Vt[:, c0 // 128 + st, :], ps)

    # ---------------- stage 2/3: per query tile ----------------
    with tc.tile_pool(name="w2", bufs=1) as wp2, \
         tc.tile_pool(name="sc", bufs=2) as scp, \
         tc.tile_pool(name="zap", bufs=1) as zpp, \
         tc.tile_pool(name="msk", bufs=2) as mkp, \
         tc.tile_pool(name="wts", bufs=2) as wtp, \
         tc.tile_pool(name="u", bufs=2) as upl, \
         tc.tile_pool(name="et", bufs=2 if STAGE == "D" else 3) as etp, \
         tc.tile_pool(name="pt", bufs=3) as ptp, \
         tc.tile_pool(name="sm", bufs=4) as smp, \
         tc.tile_pool(name="rp", bufs=1) as rpp, \
         tc.tile_pool(name="ps2", bufs=2, space="PSUM") as ps2, \
         tc.tile_pool(name="psot", bufs=3, space="PSUM") as psot, \
         tc.tile_pool(name="psD", bufs=2, space="PSUM") as psD, \
         tc.tile_pool(name="ddr", bufs=2, space="DRAM") as ddr:
        wout_sb = wp2.tile([128, 6, D], bf16)
        nc.sync.dma_start(out=wout_sb, in_=_r(woutT))
        smask_sb = wp2.tile([128, SMW], bf16)
        nc.sync.dma_start(out=smask_sb, in_=smask)

        for j in range(4):
            W = SLOT_W[j]
            side = (0, 1, 1, 0)[j]
            KT = KT_P if side == 0 else KT_Q
            Vt = V_P if side == 0 else V_Q
            kIT = kIT_P if side == 0 else kIT_Q
            NT = W // 128
            NCH = W // 512
            qs = slice(j * 128, (j + 1) * 128)

            # ---- indexer scores ----
            scores = scp.tile([128, 2048], f32, name="scores", tag="scores")
            for ch in range(NCH):
                cs = slice(ch * 512, (ch + 1) * 512)
                us = []
                for h4 in range(IH):
                    hp, t2 = h4 % 2, h4 // 2
                    ps = ps2.tile([128, 512], f32, name="ps_ix", tag="ps2")
                    nc.tensor.matmul(
                        ps, lhsT=qIT[64 * hp:64 * hp + 64, t2, qs],
                        rhs=kIT[64 * hp:64 * hp + 64, t2, cs])
                    uh = upl.tile([128, 512], f32, name="u", tag="u")
                    nc.scalar.activation(uh, ps, AF.Relu)
                    us.append(uh)
                # signed head-sum + causal/pad mask, all on gpsimd:
                # sc = smask +- u0 +- u1 +- u2 +- u3  (|w_h| folded into qIT)
                smk = smask_sb[:, SOFF[j] + ch * 512:SOFF[j] + (ch + 1) * 512]
                sc = scores[:, cs]
                if signs[0] > 0:
                    nc.gpsimd.tensor_tensor(sc, us[0], smk, AL.add)
                else:
                    nc.gpsimd.tensor_tensor(sc, smk, us[0], AL.subtract)
                for h4 in range(1, IH):
                    nc.gpsimd.tensor_tensor(
                        sc, sc, us[h4],
                        AL.add if signs[h4] > 0 else AL.subtract)

            # ---- top-64 threshold via 8x (max8 + match_replace) ----
            zap = zpp.tile([128, 2048], f32, name="zap", tag="zap")
            src = scores
            for r in range(8):
                m8 = smp.tile([128, 8], f32, name="m8", tag="m8")
                nc.vector.max(out=m8, in_=src[:, :W])
                nc.vector.match_replace(out=zap[:, :W], in_to_replace=m8,
                                        in_values=src[:, :W], imm_value=FMIN)
                src = zap
            # Masked positions carry smask(bf16)+score, not exactly FMIN; a
            # replaced masked slot would compare greater than FMIN and get
            # selected.  Clamping the zapped scores to -1e38 (far above any
            # masked value, far below any real score) kills those.
            nc.vector.tensor_scalar(zap[:, :W], zap[:, :W], -1.0e38, None,
                                    AL.max)

            m01 = mkp.tile([128, 2048], bf16, name="m01", tag="m01", bufs=1)
            nc.vector.tensor_tensor(m01[:, :W], scores[:, :W], zap[:, :W],
                                    AL.is_gt)
            wrt = mkp.tile([128, 2048], bf16, name="wrt", tag="wrt", bufs=1)
            nc.vector.tensor_tensor(wrt[:, :W], scores[:, :W], m01[:, :W],
                                    AL.mult)

            # ---- W^T and m01^T (both PE-transposed) ----
            wtsb = wtp.tile([128, 16, 128], bf16, name="wtsb", tag="wt")
            mtsb = wtp.tile([128, 16, 128], bf16, name="mtsb", tag="mt",
                            bufs=1)
            for src_t, dst in ((wrt, wtsb), (m01, mtsb)):
                for g in range((NT + 7) // 8):
                    n8 = min(8, NT - 8 * g)
                    pw = ps2.tile([128, 8, 128], bf16, name="pw", tag="pswt",
                                  bufs=1)
                    for i in range(n8):
                        st = 8 * g + i
                        nc.tensor.transpose(
                            pw[:, i, :], src_t[:, st * 128:(st + 1) * 128],
                            id_bf)
                    nc.any.tensor_copy(
                        dst[:, 8 * g:8 * g + n8, :].rearrange(
                            "p a c -> p (a c)"),
                        pw[:, :n8, :].rearrange("p a c -> p (a c)"))

            # ---- attention: PV accumulation + denominators fused ----
            # D[h, q] = sum_s exp(l)*m01^T accumulated with a ones-vector
            # matmul into a spare PSUM row of the pair's otp bank (cols
            # 128:256, partition 0 for the even head / 64 for the odd one).
            NG = NT // 4
            for he in range(0, H, 2):
                t6 = he // 2
                otp = psot.tile([128, 256], f32, name="otp", tag="psot")
                Dpair = psD.tile([128, 128], f32, name="Dpair", tag="Dp")
                for h in (he, he + 1):
                    hp = h % 2
                    pb = slice(64 * hp, 64 * hp + 64)
                    drow = Dpair[64 * hp:64 * hp + 1, :]
                    # keys x queries, [s, q] orientation -> P~T -> PV
                    for g in range(NG):
                        lt = ps2.tile([128, 512], f32, name="lt", tag="ps2")
                        ltv = lt.rearrange("p (a c) -> p a c", a=4)
                        for i in range(4):
                            st = 4 * g + i
                            nc.tensor.matmul(
                                ltv[:, i, :],
                                lhsT=KT[pb, t6, st * 128:(st + 1) * 128],
                                rhs=QT[pb, t6, qs])
                        et = etp.tile([128, 512], bf16, name="et", tag="et")
                        nc.scalar.activation(et, lt, AF.Exp, scale=0.125)
                        pt = ptp.tile([128, 512], bf16, name="pt", tag="pt")
                        nc.vector.tensor_tensor(
                            pt, et,
                            wtsb[:, 4 * g:4 * g + 4, :].rearrange(
                                "p a c -> p (a c)"), AL.mult)
                        ptv = pt.rearrange("p (a c) -> p a c", a=4)
                        jk = ptp.tile([128, 512], bf16, name="jk", tag="pt")
                        nc.vector.tensor_tensor(
                            jk, et,
                            mtsb[:, 4 * g:4 * g + 4, :].rearrange(
                                "p a c -> p (a c)"), AL.mult)
                        jkv = jk.rearrange("p (a c) -> p a c", a=4)
                        for i in range(4):
                            st = 4 * g + i
                            nc.tensor.matmul(
                                otp[pb, 0:128],
                                lhsT=Vt[:, st, h * 64:h * 64 + 64],
                                rhs=ptv[:, i, :], start=(st == 0),
                                stop=(st == NT - 1))
                            nc.tensor.matmul(
                                drow, lhsT=ones_bf, rhs=jkv[:, i, :],
                                start=(st == 0), stop=(st == NT - 1))
                if STAGE == "D":
                    dcp = smp.tile([65, 128], f32, name="dcp", tag="dcp", bufs=2)
                    nc.vector.tensor_copy(dcp[0:1, :], Dpair[0:1, :])
                    nc.vector.tensor_copy(dcp[64:65, :], Dpair[64:65, :])
                    nc.sync.dma_start(
                        out=out_d[j * 128 + 2 * t6:j * 128 + 2 * t6 + 1,
                                  0:128],
                        in_=dcp[0:1, :])
                    nc.sync.dma_start(
                        out=out_d[j * 128 + 2 * t6 + 64:
                                  j * 128 + 2 * t6 + 65, 0:128],
                        in_=dcp[64:65, :])
                # D for the pair, bounced via DRAM to broadcast each
                # head's row across the 64 partitions its PV rows occupy;
                # one full-tile reciprocal afterwards (cheap on 128 lanes)
                dD = smp.tile([65, 128], f32, name="dD", tag="ds")
                nc.scalar.copy(dD[0:1, :], Dpair[0:1, :])
                nc.scalar.copy(dD[64:65, :], Dpair[64:65, :])
                dscr0 = ddr.tile([1, 128], f32, name="dscr0", tag="dscr0")
                dscr1 = ddr.tile([1, 128], f32, name="dscr1", tag="dscr1")
                nc.sync.dma_start(out=dscr0, in_=dD[0:1, :])
                nc.sync.dma_start(out=dscr1, in_=dD[64:65, :])
                reps = rpp.tile([128, 128], f32, name="reps", tag="reps",
                                bufs=3)
                nc.sync.dma_start(out=reps[0:64, :],
                                  in_=dscr0.to_broadcast([64, 128]))
                nc.sync.dma_start(out=reps[64:128, :],
                                  in_=dscr1.to_broadcast([64, 128]))
                nc.vector.reciprocal(reps, reps)
                nc.vector.tensor_tensor(otsb[:, t6, qs], otp[:, 0:128],
                                        reps, AL.mult)

            # ---- output projection ----
            if STAGE == "D":
                continue
            for n0, nn in ((0, 512), (512, 256)):
                ops = ps2.tile([128, 512], f32, name="ops", tag="ps2")
                for kt in range(6):
                    nc.tensor.matmul(ops[:, :nn],
                                     lhsT=otsb[:, kt, qs],
                                     rhs=wout_sb[:, kt, n0:n0 + nn],
                                     start=(kt == 0), stop=(kt == 5))
                osb = smp.tile([128, 512], f32, name="osb", tag="osb", bufs=1)
                nc.any.tensor_copy(osb[:, :nn], ops[:, :nn])
                nc.sync.dma_start(out=out_d[qs, n0:n0 + nn], in_=osb[:, :nn])

    _px.close()


# ------------------------------------------------------------------
# host side
# ------------------------------------------------------------------
_CACHE = {}


def _install_ntff_hook():
    """The image lacks antenv.axon_hooks; rebuild it from trn_boot's
    ctypes NTFF profiler so run_bass_kernel_spmd(trace=True) works."""
    import sys
    import types
    if "antenv.axon_hooks" in sys.modules:
        return
    try:
        from trn_agent_boot.trn_boot import _ntff_profile_via_ctypes
        hook = _ntff_profile_via_ctypes("/opt/axon/libaxon_pjrt.so")
    except Exception:
        hook = None
    m = types.ModuleType("antenv.axon_hooks")
    m.get_axon_ntff_profile_hook = lambda: hook
    m.set_axon_ntff_profile_hook = lambda h: None
    sys.modules["antenv.axon_hooks"] = m


def make_inputs_for_core(c, x, wqkvT_bf, wiqT_s, wikT, woutT_bf):
    sm = slot_map(c)
    pbatch = sm[0][0]
    qbatch = sm[1][0]
    xT = [np.ascontiguousarray(x[b].T) for b in range(B)]
    xT_P = xT[pbatch]
    xT_Q = np.ascontiguousarray(xT[qbatch][:, :TQ])
    xTq = np.empty((D, 512), np.float32)
    smask = np.full((128, SMW), FMIN, np.float32)
    for j, (b, r, side) in enumerate(sm):
        xTq[:, j * 128:(j + 1) * 128] = xT[b][:, r * 128:(r + 1) * 128]
        Wj = SLOT_W[j]
        s = np.arange(Wj)[None, :]
        p = np.arange(128)[:, None]
        smask[:, SOFF[j]:SOFF[j] + Wj] = np.where(s <= 128 * r + p, 0.0, FMIN)
    return {
        "xT_P": xT_P, "xT_Q": xT_Q, "xTq": xTq,
        "wqkvT": wqkvT_bf, "wiqT": wiqT_s, "wikT": wikT, "woutT": woutT_bf,
        "smask": smask.astype(ml_dtypes.bfloat16),
    }


def kernel(x, wq_i, bq_i, wk_i, bk_i, w_head, w_qkv, b_qkv, w_out, b_out,
           trace=False):
    x = np.asarray(x, np.float32)
    for b_ in (bq_i, bk_i, b_qkv, b_out):
        assert np.abs(np.asarray(b_)).max() == 0.0, "nonzero bias unsupported"
    w_head = np.asarray(w_head, np.float32)
    signs = tuple(1 if s > 0 else -1 for s in w_head)

    import os
    key = (signs, os.environ.get("KSTAGE", "full"))
    if key not in _CACHE:
        _CACHE[key] = build_program(signs)
    nc = _CACHE[key]

    wqkvT_bf = np.ascontiguousarray(
        np.asarray(w_qkv, np.float32).T).astype(ml_dtypes.bfloat16)
    woutT_bf = np.ascontiguousarray(
        np.asarray(w_out, np.float32).T).astype(ml_dtypes.bfloat16)
    wiq = np.asarray(wq_i, np.float32).reshape(IH, ID, D) * \
        np.abs(w_head)[:, None, None]
    wiqT_s = np.ascontiguousarray(wiq.reshape(IH * ID, D).T)
    wikT = np.ascontiguousarray(np.asarray(wk_i, np.float32).T)

    in_maps = [make_inputs_for_core(c, x, wqkvT_bf, wiqT_s, wikT, woutT_bf)
               for c in range(NCORES)]
    kw = {}
    if trace:
        _install_ntff_hook()
        kw["trace_cores"] = list(range(NCORES))
    res = run_bass_kernel_spmd(nc, in_maps, core_ids=list(range(NCORES)),
                               trace=trace, **kw)

    out = np.empty((B, T, D), np.float32)
    for c in range(NCORES):
        oc = res.results[c]["out"]
        for j, (b, r, _s) in enumerate(slot_map(c)):
            out[b, r * 128:(r + 1) * 128, :] = oc[j * 128:(j + 1) * 128, :]
    kernel.last_result = res
    return out



# revision 31
# speedup vs baseline: 1.0495x; 1.0069x over previous
"""DeepSeek sparse attention (lightning indexer + top-64) on 8 trn2 cores.

Strategy (fully static SPMD program; per-core variation is data-only):
  - Each core owns 4 query tiles of 128 queries ("slots" with fixed key-widths
    512/1024/1536/2048).  Slot -> (batch, qtile) assignment is done on the
    host; each core uploads xT for the two batches it touches ("P" side full
    2048 cols, "Q" side first 1536 cols), the 512 query columns, and an
    additive causal/pad mask per slot.
  - Indexer path (projections + qI.kI scores) runs in fp32 (float32r matmuls)
    so the top-64 selection matches the fp32 reference bit-nearly-exactly.
  - Top-64 per query via 8 rounds of max8 + match_replace; selection becomes
    a 0/1 mask (score > zapped) and routing weights W = score * mask.
  - Attention avoids any gather: logits are computed densely in BOTH
    orientations.  [key, query] orientation: P~T = exp(L/8) * W^T feeds PV
    matmuls directly (masked positions have W == 0).  [query, key]
    orientation feeds exp + masked row-sum (tensor_tensor_reduce) to get the
    softmax denominator.  1/denominator is applied on the tiny PV output.
  - Output projection consumes the feature-major attention output, yielding
    token-major [512, 768] per core, DMA'd straight from PSUM.
"""

import numpy as np
import ml_dtypes

import concourse.bass as bass
import concourse.bacc as bacc
import concourse.mybir as mybir
import concourse.tile as tile
from concourse.bass_utils import run_bass_kernel_spmd

f32 = mybir.dt.float32
f32r = mybir.dt.float32r
bf16 = mybir.dt.bfloat16
AL = mybir.AluOpType
AF = mybir.ActivationFunctionType
AX = mybir.AxisListType

FMIN = -3.0e38
B, T, D = 2, 2048, 768
H, DH, IH, ID, TOPK = 12, 64, 4, 64, 64
TP, TQ = 2048, 1536          # key extents kept for the two batch "sides"
SLOT_W = (512, 1024, 1536, 2048)
SOFF = (0, 512, 1536, 3072)  # smask column offset per slot
SMW = 5120
NCORES = 8


def slot_map(c):
    """slot j -> (batch, qtile_index, side)   side 0 = "P", 1 = "Q"."""
    d, p = c // 2, c % 2
    q = 1 - p
    return [(p, d, 0), (q, 7 - d, 1), (q, 8 + d, 1), (p, 15 - d, 0)]


def _r(ap):
    """[768, C] dram/sbuf view -> [128, 6, C]."""
    return ap.rearrange("(a p) c -> p a c", p=128)


def build_program(signs):
    import os
    _ = os.environ.get("KSTAGE", "full")
    nc = bacc.Bacc("TRN2", target_bir_lowering=False, debug=False,
                   num_devices=NCORES)

    xT_P = nc.dram_tensor("xT_P", [D, TP], f32, kind="ExternalInput").ap()
    xT_Q = nc.dram_tensor("xT_Q", [D, TQ], f32, kind="ExternalInput").ap()
    xTq = nc.dram_tensor("xTq", [D, 512], f32, kind="ExternalInput").ap()
    wqkvT = nc.dram_tensor("wqkvT", [D, 3 * D], bf16, kind="ExternalInput").ap()
    wiqT = nc.dram_tensor("wiqT", [D, IH * ID], f32, kind="ExternalInput").ap()
    wikT = nc.dram_tensor("wikT", [D, IH * ID], f32, kind="ExternalInput").ap()
    woutT = nc.dram_tensor("woutT", [D, D], bf16, kind="ExternalInput").ap()
    smask = nc.dram_tensor("smask", [128, SMW], bf16, kind="ExternalInput").ap()
    out_d = nc.dram_tensor("out", [512, D], f32, kind="ExternalOutput").ap()

    with tile.TileContext(nc) as tc:
        _body(tc, xT_P, xT_Q, xTq, wqkvT, wiqT, wikT, woutT, smask, out_d,
              signs)
    nc.compile()
    return nc


def _body(tc, xT_P, xT_Q, xTq, wqkvT, wiqT, wikT, woutT, smask, out_d, signs):
    nc = tc.nc
    import os
    STAGE = os.environ.get("KSTAGE", "full")

    # ---------------- persistent tensors ----------------
    from contextlib import ExitStack
    _px = ExitStack()
    pp = _px.enter_context(tc.tile_pool(name="persist", bufs=1))
    KT_P = pp.tile([128, 6, TP], bf16, name="KT_P")
    KT_Q = pp.tile([128, 6, TQ], bf16, name="KT_Q")
    V_P = pp.tile([128, TP // 128, D], bf16, name="V_P")
    V_Q = pp.tile([128, TQ // 128, D], bf16, name="V_Q")
    kIT_P = pp.tile([128, 2, TP], f32r, name="kIT_P")
    kIT_Q = pp.tile([128, 2, TQ], f32r, name="kIT_Q")
    QT = pp.tile([128, 6, 512], bf16, name="QT")
    qIT = pp.tile([128, 2, 512], f32r, name="qIT")
    otsb = pp.tile([128, 6, 512], bf16, name="otsb")
    id_bf = pp.tile([128, 128], bf16, name="id_bf")
    ones_bf = pp.tile([128, 1], bf16, name="ones_bf")

    # identity matrix for PE transposes
    nc.vector.memset(id_bf, 1.0)
    nc.gpsimd.affine_select(id_bf, id_bf, pattern=[[-1, 128]],
                            compare_op=AL.is_equal, fill=0.0, base=0,
                            channel_multiplier=1)
    nc.vector.memset(ones_bf, 1.0)

    # ---------------- stage 1: projections ----------------
    CH = 256
    with tc.tile_pool(name="w1", bufs=1) as wp, \
         tc.tile_pool(name="ps1", bufs=3, space="PSUM") as ps1, \
         tc.tile_pool(name="psv", bufs=2, space="PSUM") as psv:
        wqkv_sb = wp.tile([128, 6, 3 * D], bf16)
        wiq_sb = wp.tile([128, 6, IH * ID], f32)
        wik_sb = wp.tile([128, 6, IH * ID], f32)
        wqv = _r(wqkvT)
        for a in range(6):
            nc.sync.dma_start(out=wqkv_sb[:, a, :], in_=wqv[:, a, :])
        wiv = _r(wiqT)
        wkv = _r(wikT)
        for a in range(0, 6, 3):
            nc.sync.dma_start(out=wiq_sb[:, a:a + 3, :],
                              in_=wiv[:, a:a + 3, :])
            nc.sync.dma_start(out=wik_sb[:, a:a + 3, :],
                              in_=wkv[:, a:a + 3, :])
        # fp32r operands must come from an op that rounds to fp32r
        wiq_r = wp.tile([128, 6, IH * ID], f32r)
        wik_r = wp.tile([128, 6, IH * ID], f32r)
        nc.any.tensor_copy(wiq_r.rearrange("p a c -> p (a c)"),
                           wiq_sb.rearrange("p a c -> p (a c)"))
        nc.any.tensor_copy(wik_r.rearrange("p a c -> p (a c)"),
                           wik_sb.rearrange("p a c -> p (a c)"))

        # queries first (they are needed by every slot)
        with tc.tile_pool(name="xq", bufs=1) as xqp:
            xqv = _r(xTq)
            for q0 in range(0, 512, 256):
                xq = xqp.tile([128, 6, 256], f32, name="xq", tag="xq")
                for a in range(0, 6, 3):
                    nc.sync.dma_start(out=xq[:, a:a + 3, :],
                                      in_=xqv[:, a:a + 3, q0:q0 + 256])
                xqb = xqp.tile([128, 6, 256], bf16, name="xqb", tag="xqb")
                nc.any.tensor_copy(xqb.rearrange("p a c -> p (a c)"),
                                   xq.rearrange("p a c -> p (a c)"))
                xqr = xqp.tile([128, 6, 256], f32r, name="xqr", tag="xqr")
                nc.any.tensor_copy(xqr.rearrange("p a c -> p (a c)"),
                                   xq.rearrange("p a c -> p (a c)"))
                for mt in range(2):
                    ps = ps1.tile([128, 256], f32, name="ps_qi", tag="ps1")
                    for kt in range(6):
                        nc.tensor.matmul(
                            ps, lhsT=wiq_r[:, kt, mt * 128:(mt + 1) * 128],
                            rhs=xqr[:, kt, :], start=(kt == 0), stop=(kt == 5))
                    nc.any.tensor_copy(qIT[:, mt, q0:q0 + 256], ps)
                for mt in range(6):
                    ps = ps1.tile([128, 256], f32, name="ps_qt", tag="ps1")
                    for kt in range(6):
                        nc.tensor.matmul(
                            ps, lhsT=wqkv_sb[:, kt, mt * 128:(mt + 1) * 128],
                            rhs=xqb[:, kt, :], start=(kt == 0), stop=(kt == 5))
                    nc.any.tensor_copy(QT[:, mt, q0:q0 + 256], ps)

        with tc.tile_pool(name="xc", bufs=2) as xcp, \
             tc.tile_pool(name="xcb", bufs=2) as xbp:
            for side, (xT, S) in enumerate(((xT_P, TP), (xT_Q, TQ))):
                xv = _r(xT)
                KT = KT_P if side == 0 else KT_Q
                Vt = V_P if side == 0 else V_Q
                kIT = kIT_P if side == 0 else kIT_Q
                for c0 in range(0, S, CH):
                    xc = xcp.tile([128, 6, CH], f32, name="xc", tag="xc")
                    for a in range(0, 6, 3):
                        nc.sync.dma_start(out=xc[:, a:a + 3, :],
                                          in_=xv[:, a:a + 3, c0:c0 + CH])
                    xcb = xbp.tile([128, 6, CH], bf16, name="xcb", tag="xcb")
                    nc.any.tensor_copy(xcb.rearrange("p a c -> p (a c)"),
                                       xc.rearrange("p a c -> p (a c)"))
                    xcr = xbp.tile([128, 6, CH], f32r, name="xcr", tag="xcr", bufs=1)
                    nc.any.tensor_copy(xcr.rearrange("p a c -> p (a c)"),
                                       xc.rearrange("p a c -> p (a c)"))
                    # indexer keys (fp32r, exact)
                    for mt in range(2):
                        ps = ps1.tile([128, CH], f32, name="ps_ki", tag="ps1")
                        for kt in range(6):
                            nc.tensor.matmul(
                                ps, lhsT=wik_r[:, kt, mt * 128:(mt + 1) * 128],
                                rhs=xcr[:, kt, :],
                                start=(kt == 0), stop=(kt == 5))
                        nc.any.tensor_copy(kIT[:, mt, c0:c0 + CH], ps)
                    # attention keys KT (bf16)
                    for mt in range(6):
                        ps = ps1.tile([128, CH], f32, name="ps_kt", tag="ps1")
                        for kt in range(6):
                            nc.tensor.matmul(
                                ps,
                                lhsT=wqkv_sb[:, kt,
                                             D + mt * 128:D + (mt + 1) * 128],
                                rhs=xcb[:, kt, :], start=(kt == 0),
                                stop=(kt == 5))
                        nc.any.tensor_copy(KT[:, mt, c0:c0 + CH], ps)
                    # values V (token-major, bf16)
                    for st in range(CH // 128):
                        ps = psv.tile([128, D], f32, name="ps_v", tag="psv")
                        for n0, nn in ((0, 512), (512, 256)):
                            for kt in range(6):
                                nc.tensor.matmul(
                                    ps[:, n0:n0 + nn],
                                    lhsT=xcb[:, kt, st * 128:(st + 1) * 128],
                                    rhs=wqkv_sb[:, kt,
                                                2 * D + n0:2 * D + n0 + nn],
                                    start=(kt == 0), stop=(kt == 5))
                        nc.any.tensor_copy(Vt[:, c0 // 128 + st, :], ps)

    # ---------------- stage 2/3: per query tile ----------------
    with tc.tile_pool(name="w2", bufs=1) as wp2, \
         tc.tile_pool(name="sc", bufs=2) as scp, \
         tc.tile_pool(name="zap", bufs=1) as zpp, \
         tc.tile_pool(name="msk", bufs=2) as mkp, \
         tc.tile_pool(name="wts", bufs=2) as wtp, \
         tc.tile_pool(name="u", bufs=2) as upl, \
         tc.tile_pool(name="et", bufs=2 if STAGE == "D" else 3) as etp, \
         tc.tile_pool(name="pt", bufs=3) as ptp, \
         tc.tile_pool(name="sm", bufs=4) as smp, \
         tc.tile_pool(name="rp", bufs=1) as rpp, \
         tc.tile_pool(name="ps2", bufs=2, space="PSUM") as ps2, \
         tc.tile_pool(name="psot", bufs=3, space="PSUM") as psot, \
         tc.tile_pool(name="psD", bufs=2, space="PSUM") as psD, \
         tc.tile_pool(name="ddr", bufs=2, space="DRAM") as ddr:
        wout_sb = wp2.tile([128, 6, D], bf16)
        wov = _r(woutT)
        for a in range(0, 6, 3):
            nc.sync.dma_start(out=wout_sb[:, a:a + 3, :],
                              in_=wov[:, a:a + 3, :])
        smask_sb = wp2.tile([128, SMW], bf16)
        nc.sync.dma_start(out=smask_sb, in_=smask)

        for j in range(4):
            W = SLOT_W[j]
            side = (0, 1, 1, 0)[j]
            KT = KT_P if side == 0 else KT_Q
            Vt = V_P if side == 0 else V_Q
            kIT = kIT_P if side == 0 else kIT_Q
            NT = W // 128
            NCH = W // 512
            qs = slice(j * 128, (j + 1) * 128)

            # ---- indexer scores ----
            scores = scp.tile([128, 2048], f32, name="scores", tag="scores")
            for ch in range(NCH):
                cs = slice(ch * 512, (ch + 1) * 512)
                us = []
                for h4 in range(IH):
                    hp, t2 = h4 % 2, h4 // 2
                    ps = ps2.tile([128, 512], f32, name="ps_ix", tag="ps2")
                    nc.tensor.matmul(
                        ps, lhsT=qIT[64 * hp:64 * hp + 64, t2, qs],
                        rhs=kIT[64 * hp:64 * hp + 64, t2, cs])
                    uh = upl.tile([128, 512], f32, name="u", tag="u")
                    nc.scalar.activation(uh, ps, AF.Relu)
                    us.append(uh)
                # signed head-sum + causal/pad mask, all on gpsimd:
                # sc = smask +- u0 +- u1 +- u2 +- u3  (|w_h| folded into qIT)
                smk = smask_sb[:, SOFF[j] + ch * 512:SOFF[j] + (ch + 1) * 512]
                sc = scores[:, cs]
                if signs[0] > 0:
                    nc.gpsimd.tensor_tensor(sc, us[0], smk, AL.add)
                else:
                    nc.gpsimd.tensor_tensor(sc, smk, us[0], AL.subtract)
                for h4 in range(1, IH):
                    nc.gpsimd.tensor_tensor(
                        sc, sc, us[h4],
                        AL.add if signs[h4] > 0 else AL.subtract)

            # ---- top-64 threshold via 8x (max8 + match_replace) ----
            zap = zpp.tile([128, 2048], f32, name="zap", tag="zap")
            src = scores
            for r in range(8):
                m8 = smp.tile([128, 8], f32, name="m8", tag="m8")
                nc.vector.max(out=m8, in_=src[:, :W])
                nc.vector.match_replace(out=zap[:, :W], in_to_replace=m8,
                                        in_values=src[:, :W], imm_value=FMIN)
                src = zap
            # Masked positions carry smask(bf16)+score, not exactly FMIN; a
            # replaced masked slot would compare greater than FMIN and get
            # selected.  Clamping the zapped scores to -1e38 (far above any
            # masked value, far below any real score) kills those.
            nc.vector.tensor_scalar(zap[:, :W], zap[:, :W], -1.0e38, None,
                                    AL.max)

            m01 = mkp.tile([128, 2048], bf16, name="m01", tag="m01", bufs=1)
            nc.vector.tensor_tensor(m01[:, :W], scores[:, :W], zap[:, :W],
                                    AL.is_gt)
            wrt = mkp.tile([128, 2048], bf16, name="wrt", tag="wrt", bufs=1)
            nc.vector.tensor_tensor(wrt[:, :W], scores[:, :W], m01[:, :W],
                                    AL.mult)

            # ---- W^T and m01^T (both PE-transposed) ----
            wtsb = wtp.tile([128, 16, 128], bf16, name="wtsb", tag="wt")
            mtsb = wtp.tile([128, 16, 128], bf16, name="mtsb", tag="mt",
                            bufs=1)
            for src_t, dst in ((wrt, wtsb), (m01, mtsb)):
                for g in range((NT + 7) // 8):
                    n8 = min(8, NT - 8 * g)
                    pw = ps2.tile([128, 8, 128], bf16, name="pw", tag="pswt",
                                  bufs=1)
                    for i in range(n8):
                        st = 8 * g + i
                        nc.tensor.transpose(
                            pw[:, i, :], src_t[:, st * 128:(st + 1) * 128],
                            id_bf)
                    nc.any.tensor_copy(
                        dst[:, 8 * g:8 * g + n8, :].rearrange(
                            "p a c -> p (a c)"),
                        pw[:, :n8, :].rearrange("p a c -> p (a c)"))

            # ---- attention: PV accumulation + denominators fused ----
            # D[h, q] = sum_s exp(l)*m01^T accumulated with a ones-vector
            # matmul into a spare PSUM row of the pair's otp bank (cols
            # 128:256, partition 0 for the even head / 64 for the odd one).
            NG = NT // 4
            for he in range(0, H, 2):
                t6 = he // 2
                otp = psot.tile([128, 256], f32, name="otp", tag="psot")
                Dpair = psD.tile([128, 128], f32, name="Dpair", tag="Dp")
                for h in (he, he + 1):
                    hp = h % 2
                    pb = slice(64 * hp, 64 * hp + 64)
                    drow = Dpair[64 * hp:64 * hp + 1, :]
                    # keys x queries, [s, q] orientation -> P~T -> PV
                    for g in range(NG):
                        lt = ps2.tile([128, 512], f32, name="lt", tag="ps2")
                        ltv = lt.rearrange("p (a c) -> p a c", a=4)
                        for i in range(4):
                            st = 4 * g + i
                            nc.tensor.matmul(
                                ltv[:, i, :],
                                lhsT=KT[pb, t6, st * 128:(st + 1) * 128],
                                rhs=QT[pb, t6, qs])
                        et = etp.tile([128, 512], bf16, name="et", tag="et")
                        nc.scalar.activation(et, lt, AF.Exp, scale=0.125)
                        pt = ptp.tile([128, 512], bf16, name="pt", tag="pt")
                        nc.vector.tensor_tensor(
                            pt, et,
                            wtsb[:, 4 * g:4 * g + 4, :].rearrange(
                                "p a c -> p (a c)"), AL.mult)
                        ptv = pt.rearrange("p (a c) -> p a c", a=4)
                        jk = ptp.tile([128, 512], bf16, name="jk", tag="pt")
                        nc.vector.tensor_tensor(
                            jk, et,
                            mtsb[:, 4 * g:4 * g + 4, :].rearrange(
                                "p a c -> p (a c)"), AL.mult)
                        jkv = jk.rearrange("p (a c) -> p a c", a=4)
                        for i in range(4):
                            st = 4 * g + i
                            nc.tensor.matmul(
                                otp[pb, 0:128],
                                lhsT=Vt[:, st, h * 64:h * 64 + 64],
                                rhs=ptv[:, i, :], start=(st == 0),
                                stop=(st == NT - 1))
                            nc.tensor.matmul(
                                drow, lhsT=ones_bf, rhs=jkv[:, i, :],
                                start=(st == 0), stop=(st == NT - 1))
                if STAGE == "D":
                    dcp = smp.tile([65, 128], f32, name="dcp", tag="dcp", bufs=2)
                    nc.vector.tensor_copy(dcp[0:1, :], Dpair[0:1, :])
                    nc.vector.tensor_copy(dcp[64:65, :], Dpair[64:65, :])
                    nc.sync.dma_start(
                        out=out_d[j * 128 + 2 * t6:j * 128 + 2 * t6 + 1,
                                  0:128],
                        in_=dcp[0:1, :])
                    nc.sync.dma_start(
                        out=out_d[j * 128 + 2 * t6 + 64:
                                  j * 128 + 2 * t6 + 65, 0:128],
                        in_=dcp[64:65, :])
                # D for the pair, bounced via DRAM to broadcast each
                # head's row across the 64 partitions its PV rows occupy;
                # one full-tile reciprocal afterwards (cheap on 128 lanes)
                dD = smp.tile([65, 128], f32, name="dD", tag="ds")
                nc.scalar.copy(dD[0:1, :], Dpair[0:1, :])
                nc.scalar.copy(dD[64:65, :], Dpair[64:65, :])
                dscr0 = ddr.tile([1, 128], f32, name="dscr0", tag="dscr0")
                dscr1 = ddr.tile([1, 128], f32, name="dscr1", tag="dscr1")
                nc.sync.dma_start(out=dscr0, in_=dD[0:1, :])
                nc.sync.dma_start(out=dscr1, in_=dD[64:65, :])
                reps = rpp.tile([128, 128], f32, name="reps", tag="reps",
                                bufs=3)
                nc.sync.dma_start(out=reps[0:64, :],
                                  in_=dscr0.to_broadcast([64, 128]))
                nc.sync.dma_start(out=reps[64:128, :],
                                  in_=dscr1.to_broadcast([64, 128]))
                nc.vector.reciprocal(reps, reps)
                nc.vector.tensor_tensor(otsb[:, t6, qs], otp[:, 0:128],
                                        reps, AL.mult)

            # ---- output projection ----
            if STAGE == "D":
                continue
            for n0, nn in ((0, 512), (512, 256)):
                ops = ps2.tile([128, 512], f32, name="ops", tag="ps2")
                for kt in range(6):
                    nc.tensor.matmul(ops[:, :nn],
                                     lhsT=otsb[:, kt, qs],
                                     rhs=wout_sb[:, kt, n0:n0 + nn],
                                     start=(kt == 0), stop=(kt == 5))
                osb = smp.tile([128, 512], f32, name="osb", tag="osb", bufs=1)
                nc.any.tensor_copy(osb[:, :nn], ops[:, :nn])
                nc.sync.dma_start(out=out_d[qs, n0:n0 + nn], in_=osb[:, :nn])

    _px.close()


# ------------------------------------------------------------------
# host side
# ------------------------------------------------------------------
_CACHE = {}


def _install_ntff_hook():
    """The image lacks antenv.axon_hooks; rebuild it from trn_boot's
    ctypes NTFF profiler so run_bass_kernel_spmd(trace=True) works."""
    import sys
    import types
    if "antenv.axon_hooks" in sys.modules:
        return
    try:
        from trn_agent_boot.trn_boot import _ntff_profile_via_ctypes
        hook = _ntff_profile_via_ctypes("/opt/axon/libaxon_pjrt.so")
    except Exception:
        hook = None
    m = types.ModuleType("antenv.axon_hooks")
    m.get_axon_ntff_profile_hook = lambda: hook
    m.set_axon_ntff_profile_hook = lambda h: None
    sys.modules["antenv.axon_hooks"] = m


def make_inputs_for_core(c, x, wqkvT_bf, wiqT_s, wikT, woutT_bf):
    sm = slot_map(c)
    pbatch = sm[0][0]
    qbatch = sm[1][0]
    xT = [np.ascontiguousarray(x[b].T) for b in range(B)]
    xT_P = xT[pbatch]
    xT_Q = np.ascontiguousarray(xT[qbatch][:, :TQ])
    xTq = np.empty((D, 512), np.float32)
    smask = np.full((128, SMW), FMIN, np.float32)
    for j, (b, r, side) in enumerate(sm):
        xTq[:, j * 128:(j + 1) * 128] = xT[b][:, r * 128:(r + 1) * 128]
        Wj = SLOT_W[j]
        s = np.arange(Wj)[None, :]
        p = np.arange(128)[:, None]
        smask[:, SOFF[j]:SOFF[j] + Wj] = np.where(s <= 128 * r + p, 0.0, FMIN)
    return {
        "xT_P": xT_P, "xT_Q": xT_Q, "xTq": xTq,
        "wqkvT": wqkvT_bf, "wiqT": wiqT_s, "wikT": wikT, "woutT": woutT_bf,
        "smask": smask.astype(ml_dtypes.bfloat16),
    }


def kernel(x, wq_i, bq_i, wk_i, bk_i, w_head, w_qkv, b_qkv, w_out, b_out,
           trace=False):
    x = np.asarray(x, np.float32)
    for b_ in (bq_i, bk_i, b_qkv, b_out):
        assert np.abs(np.asarray(b_)).max() == 0.0, "nonzero bias unsupported"
    w_head = np.asarray(w_head, np.float32)
    signs = tuple(1 if s > 0 else -1 for s in w_head)

    import os
    key = (signs, os.environ.get("KSTAGE", "full"))
    if key not in _CACHE:
        _CACHE[key] = build_program(signs)
    nc = _CACHE[key]

    wqkvT_bf = np.ascontiguousarray(
        np.asarray(w_qkv, np.float32).T).astype(ml_dtypes.bfloat16)
    woutT_bf = np.ascontiguousarray(
        np.asarray(w_out, np.float32).T).astype(ml_dtypes.bfloat16)
    wiq = np.asarray(wq_i, np.float32).reshape(IH, ID, D) * \
        np.abs(w_head)[:, None, None]
    wiqT_s = np.ascontiguousarray(wiq.reshape(IH * ID, D).T)
    wikT = np.ascontiguousarray(np.asarray(wk_i, np.float32).T)

    in_maps = [make_inputs_for_core(c, x, wqkvT_bf, wiqT_s, wikT, woutT_bf)
               for c in range(NCORES)]
    kw = {}
    if trace:
        _install_ntff_hook()
        kw["trace_cores"] = list(range(NCORES))
    res = run_bass_kernel_spmd(nc, in_maps, core_ids=list(range(NCORES)),
                               trace=trace, **kw)

    out = np.empty((B, T, D), np.float32)
    for c in range(NCORES):
        oc = res.results[c]["out"]
        for j, (b, r, _s) in enumerate(slot_map(c)):
            out[b, r * 128:(r + 1) * 128, :] = oc[j * 128:(j + 1) * 128, :]
    kernel.last_result = res
    return out



# revision 32
# speedup vs baseline: 1.0592x; 1.0092x over previous
"""DeepSeek sparse attention (lightning indexer + top-64) on 8 trn2 cores.

Strategy (fully static SPMD program; per-core variation is data-only):
  - Each core owns 4 query tiles of 128 queries ("slots" with fixed key-widths
    512/1024/1536/2048).  Slot -> (batch, qtile) assignment is done on the
    host; each core uploads xT for the two batches it touches ("P" side full
    2048 cols, "Q" side first 1536 cols), the 512 query columns, and an
    additive causal/pad mask per slot.
  - Indexer path (projections + qI.kI scores) runs in fp32 (float32r matmuls)
    so the top-64 selection matches the fp32 reference bit-nearly-exactly.
  - Top-64 per query via 8 rounds of max8 + match_replace; selection becomes
    a 0/1 mask (score > zapped) and routing weights W = score * mask.
  - Attention avoids any gather: logits are computed densely in BOTH
    orientations.  [key, query] orientation: P~T = exp(L/8) * W^T feeds PV
    matmuls directly (masked positions have W == 0).  [query, key]
    orientation feeds exp + masked row-sum (tensor_tensor_reduce) to get the
    softmax denominator.  1/denominator is applied on the tiny PV output.
  - Output projection consumes the feature-major attention output, yielding
    token-major [512, 768] per core, DMA'd straight from PSUM.
"""

import numpy as np
import ml_dtypes

import concourse.bass as bass
import concourse.bacc as bacc
import concourse.mybir as mybir
import concourse.tile as tile
from concourse.bass_utils import run_bass_kernel_spmd

f32 = mybir.dt.float32
f32r = mybir.dt.float32r
bf16 = mybir.dt.bfloat16
AL = mybir.AluOpType
AF = mybir.ActivationFunctionType
AX = mybir.AxisListType

FMIN = -3.0e38
B, T, D = 2, 2048, 768
H, DH, IH, ID, TOPK = 12, 64, 4, 64, 64
TP, TQ = 2048, 1536          # key extents kept for the two batch "sides"
SLOT_W = (512, 1024, 1536, 2048)
SOFF = (0, 512, 1536, 3072)  # smask column offset per slot
SMW = 5120
NCORES = 8


def slot_map(c):
    """slot j -> (batch, qtile_index, side)   side 0 = "P", 1 = "Q"."""
    d, p = c // 2, c % 2
    q = 1 - p
    return [(p, d, 0), (q, 7 - d, 1), (q, 8 + d, 1), (p, 15 - d, 0)]


def _r(ap):
    """[768, C] dram/sbuf view -> [128, 6, C]."""
    return ap.rearrange("(a p) c -> p a c", p=128)


def build_program(signs):
    import os
    _ = os.environ.get("KSTAGE", "full")
    nc = bacc.Bacc("TRN2", target_bir_lowering=False, debug=False,
                   num_devices=NCORES)

    xT_P = nc.dram_tensor("xT_P", [D, TP], f32, kind="ExternalInput").ap()
    xT_Q = nc.dram_tensor("xT_Q", [D, TQ], f32, kind="ExternalInput").ap()
    xTq = nc.dram_tensor("xTq", [D, 512], f32, kind="ExternalInput").ap()
    wqkvT = nc.dram_tensor("wqkvT", [D, 3 * D], bf16, kind="ExternalInput").ap()
    wiqT = nc.dram_tensor("wiqT", [D, IH * ID], f32, kind="ExternalInput").ap()
    wikT = nc.dram_tensor("wikT", [D, IH * ID], f32, kind="ExternalInput").ap()
    woutT = nc.dram_tensor("woutT", [D, D], bf16, kind="ExternalInput").ap()
    smask = nc.dram_tensor("smask", [128, SMW], bf16, kind="ExternalInput").ap()
    out_d = nc.dram_tensor("out", [512, D], f32, kind="ExternalOutput").ap()

    with tile.TileContext(nc) as tc:
        _body(tc, xT_P, xT_Q, xTq, wqkvT, wiqT, wikT, woutT, smask, out_d,
              signs)
    nc.compile()
    return nc


def _body(tc, xT_P, xT_Q, xTq, wqkvT, wiqT, wikT, woutT, smask, out_d, signs):
    nc = tc.nc
    import os
    STAGE = os.environ.get("KSTAGE", "full")

    # ---------------- persistent tensors ----------------
    from contextlib import ExitStack
    _px = ExitStack()
    pp = _px.enter_context(tc.tile_pool(name="persist", bufs=1))
    KT_P = pp.tile([128, 6, TP], bf16, name="KT_P")
    KT_Q = pp.tile([128, 6, TQ], bf16, name="KT_Q")
    V_P = pp.tile([128, TP // 128, D], bf16, name="V_P")
    V_Q = pp.tile([128, TQ // 128, D], bf16, name="V_Q")
    kIT_P = pp.tile([128, 2, TP], f32r, name="kIT_P")
    kIT_Q = pp.tile([128, 2, TQ], f32r, name="kIT_Q")
    QT = pp.tile([128, 6, 512], bf16, name="QT")
    qIT = pp.tile([128, 2, 512], f32r, name="qIT")
    otsb = pp.tile([128, 6, 512], bf16, name="otsb")
    id_bf = pp.tile([128, 128], bf16, name="id_bf")
    ones_bf = pp.tile([128, 1], bf16, name="ones_bf")

    # identity matrix for PE transposes
    nc.vector.memset(id_bf, 1.0)
    nc.gpsimd.affine_select(id_bf, id_bf, pattern=[[-1, 128]],
                            compare_op=AL.is_equal, fill=0.0, base=0,
                            channel_multiplier=1)
    nc.vector.memset(ones_bf, 1.0)

    # ---------------- stage 1: projections ----------------
    CH = 256
    with tc.tile_pool(name="w1", bufs=1) as wp, \
         tc.tile_pool(name="ps1", bufs=3, space="PSUM") as ps1, \
         tc.tile_pool(name="psv", bufs=2, space="PSUM") as psv:
        wqkv_sb = wp.tile([128, 6, 3 * D], bf16)
        wiq_sb = wp.tile([128, 6, IH * ID], f32)
        wik_sb = wp.tile([128, 6, IH * ID], f32)
        wqv = _r(wqkvT)
        for a in range(6):
            nc.sync.dma_start(out=wqkv_sb[:, a, :], in_=wqv[:, a, :])
        wiv = _r(wiqT)
        wkv = _r(wikT)
        for a in range(0, 6, 3):
            nc.sync.dma_start(out=wiq_sb[:, a:a + 3, :],
                              in_=wiv[:, a:a + 3, :])
            nc.sync.dma_start(out=wik_sb[:, a:a + 3, :],
                              in_=wkv[:, a:a + 3, :])
        # fp32r operands must come from an op that rounds to fp32r
        wiq_r = wp.tile([128, 6, IH * ID], f32r)
        wik_r = wp.tile([128, 6, IH * ID], f32r)
        nc.any.tensor_copy(wiq_r.rearrange("p a c -> p (a c)"),
                           wiq_sb.rearrange("p a c -> p (a c)"))
        nc.any.tensor_copy(wik_r.rearrange("p a c -> p (a c)"),
                           wik_sb.rearrange("p a c -> p (a c)"))

        # queries first (they are needed by every slot)
        with tc.tile_pool(name="xq", bufs=1) as xqp:
            xqv = _r(xTq)
            for q0 in range(0, 512, 256):
                xq = xqp.tile([128, 6, 256], f32, name="xq", tag="xq")
                for a in range(0, 6, 3):
                    nc.sync.dma_start(out=xq[:, a:a + 3, :],
                                      in_=xqv[:, a:a + 3, q0:q0 + 256])
                xqb = xqp.tile([128, 6, 256], bf16, name="xqb", tag="xqb")
                nc.any.tensor_copy(xqb.rearrange("p a c -> p (a c)"),
                                   xq.rearrange("p a c -> p (a c)"))
                xqr = xqp.tile([128, 6, 256], f32r, name="xqr", tag="xqr")
                nc.any.tensor_copy(xqr.rearrange("p a c -> p (a c)"),
                                   xq.rearrange("p a c -> p (a c)"))
                for mt in range(2):
                    ps = ps1.tile([128, 256], f32, name="ps_qi", tag="ps1")
                    for kt in range(6):
                        nc.tensor.matmul(
                            ps, lhsT=wiq_r[:, kt, mt * 128:(mt + 1) * 128],
                            rhs=xqr[:, kt, :], start=(kt == 0), stop=(kt == 5))
                    nc.any.tensor_copy(qIT[:, mt, q0:q0 + 256], ps)
                for mt in range(6):
                    ps = ps1.tile([128, 256], f32, name="ps_qt", tag="ps1")
                    for kt in range(6):
                        nc.tensor.matmul(
                            ps, lhsT=wqkv_sb[:, kt, mt * 128:(mt + 1) * 128],
                            rhs=xqb[:, kt, :], start=(kt == 0), stop=(kt == 5))
                    nc.any.tensor_copy(QT[:, mt, q0:q0 + 256], ps)

        with tc.tile_pool(name="xc", bufs=2) as xcp, \
             tc.tile_pool(name="xcb", bufs=2) as xbp:
            for side, (xT, S) in enumerate(((xT_P, TP), (xT_Q, TQ))):
                xv = _r(xT)
                KT = KT_P if side == 0 else KT_Q
                Vt = V_P if side == 0 else V_Q
                kIT = kIT_P if side == 0 else kIT_Q
                for c0 in range(0, S, CH):
                    xc = xcp.tile([128, 6, CH], f32, name="xc", tag="xc")
                    for a in range(0, 6, 3):
                        nc.sync.dma_start(out=xc[:, a:a + 3, :],
                                          in_=xv[:, a:a + 3, c0:c0 + CH])
                    xcb = xbp.tile([128, 6, CH], bf16, name="xcb", tag="xcb")
                    nc.any.tensor_copy(xcb.rearrange("p a c -> p (a c)"),
                                       xc.rearrange("p a c -> p (a c)"))
                    xcr = xbp.tile([128, 6, CH], f32r, name="xcr", tag="xcr", bufs=1)
                    nc.any.tensor_copy(xcr.rearrange("p a c -> p (a c)"),
                                       xc.rearrange("p a c -> p (a c)"))
                    # indexer keys (fp32r, exact)
                    for mt in range(2):
                        ps = ps1.tile([128, CH], f32, name="ps_ki", tag="ps1")
                        for kt in range(6):
                            nc.tensor.matmul(
                                ps, lhsT=wik_r[:, kt, mt * 128:(mt + 1) * 128],
                                rhs=xcr[:, kt, :],
                                start=(kt == 0), stop=(kt == 5))
                        nc.any.tensor_copy(kIT[:, mt, c0:c0 + CH], ps)
                    # attention keys KT (bf16)
                    for mt in range(6):
                        ps = ps1.tile([128, CH], f32, name="ps_kt", tag="ps1")
                        for kt in range(6):
                            nc.tensor.matmul(
                                ps,
                                lhsT=wqkv_sb[:, kt,
                                             D + mt * 128:D + (mt + 1) * 128],
                                rhs=xcb[:, kt, :], start=(kt == 0),
                                stop=(kt == 5))
                        nc.any.tensor_copy(KT[:, mt, c0:c0 + CH], ps)
                    # values V (token-major, bf16)
                    for st in range(CH // 128):
                        ps = psv.tile([128, D], f32, name="ps_v", tag="psv")
                        for n0, nn in ((0, 512), (512, 256)):
                            for kt in range(6):
                                nc.tensor.matmul(
                                    ps[:, n0:n0 + nn],
                                    lhsT=xcb[:, kt, st * 128:(st + 1) * 128],
                                    rhs=wqkv_sb[:, kt,
                                                2 * D + n0:2 * D + n0 + nn],
                                    start=(kt == 0), stop=(kt == 5))
                        nc.any.tensor_copy(Vt[:, c0 // 128 + st, :], ps)

    # ---------------- stage 2/3: per query tile ----------------
    with tc.tile_pool(name="w2", bufs=1) as wp2, \
         tc.tile_pool(name="sc", bufs=2) as scp, \
         tc.tile_pool(name="zap", bufs=1) as zpp, \
         tc.tile_pool(name="msk", bufs=2) as mkp, \
         tc.tile_pool(name="wts", bufs=2) as wtp, \
         tc.tile_pool(name="u", bufs=2) as upl, \
         tc.tile_pool(name="et", bufs=2 if STAGE == "D" else 3) as etp, \
         tc.tile_pool(name="pt", bufs=3) as ptp, \
         tc.tile_pool(name="sm", bufs=4) as smp, \
         tc.tile_pool(name="rp", bufs=1) as rpp, \
         tc.tile_pool(name="ps2", bufs=2, space="PSUM") as ps2, \
         tc.tile_pool(name="psot", bufs=2, space="PSUM") as psot, \
         tc.tile_pool(name="psD", bufs=2, space="PSUM") as psD, \
         tc.tile_pool(name="ddr", bufs=2, space="DRAM") as ddr:
        wout_sb = wp2.tile([128, 6, D], bf16)
        wov = _r(woutT)
        for a in range(0, 6, 3):
            nc.sync.dma_start(out=wout_sb[:, a:a + 3, :],
                              in_=wov[:, a:a + 3, :])
        smask_sb = wp2.tile([128, SMW], bf16)
        nc.sync.dma_start(out=smask_sb, in_=smask)

        for j in range(4):
            W = SLOT_W[j]
            side = (0, 1, 1, 0)[j]
            KT = KT_P if side == 0 else KT_Q
            Vt = V_P if side == 0 else V_Q
            kIT = kIT_P if side == 0 else kIT_Q
            NT = W // 128
            NCH = W // 512
            qs = slice(j * 128, (j + 1) * 128)

            # ---- indexer scores ----
            scores = scp.tile([128, 2048], f32, name="scores", tag="scores")
            for ch in range(NCH):
                cs = slice(ch * 512, (ch + 1) * 512)
                us = []
                for h4 in range(IH):
                    hp, t2 = h4 % 2, h4 // 2
                    ps = ps2.tile([128, 512], f32, name="ps_ix", tag="ps2")
                    nc.tensor.matmul(
                        ps, lhsT=qIT[64 * hp:64 * hp + 64, t2, qs],
                        rhs=kIT[64 * hp:64 * hp + 64, t2, cs])
                    uh = upl.tile([128, 512], f32, name="u", tag="u")
                    nc.scalar.activation(uh, ps, AF.Relu)
                    us.append(uh)
                # signed head-sum + causal/pad mask, all on gpsimd:
                # sc = smask +- u0 +- u1 +- u2 +- u3  (|w_h| folded into qIT)
                smk = smask_sb[:, SOFF[j] + ch * 512:SOFF[j] + (ch + 1) * 512]
                sc = scores[:, cs]
                if signs[0] > 0:
                    nc.gpsimd.tensor_tensor(sc, us[0], smk, AL.add)
                else:
                    nc.gpsimd.tensor_tensor(sc, smk, us[0], AL.subtract)
                for h4 in range(1, IH):
                    nc.gpsimd.tensor_tensor(
                        sc, sc, us[h4],
                        AL.add if signs[h4] > 0 else AL.subtract)

            # ---- top-64 threshold via 8x (max8 + match_replace) ----
            zap = zpp.tile([128, 2048], f32, name="zap", tag="zap")
            src = scores
            for r in range(8):
                m8 = smp.tile([128, 8], f32, name="m8", tag="m8")
                nc.vector.max(out=m8, in_=src[:, :W])
                nc.vector.match_replace(out=zap[:, :W], in_to_replace=m8,
                                        in_values=src[:, :W], imm_value=FMIN)
                src = zap
            # Masked positions carry smask(bf16)+score, not exactly FMIN; a
            # replaced masked slot would compare greater than FMIN and get
            # selected.  Clamping the zapped scores to -1e38 (far above any
            # masked value, far below any real score) kills those.
            nc.vector.tensor_scalar(zap[:, :W], zap[:, :W], -1.0e38, None,
                                    AL.max)

            m01 = mkp.tile([128, 2048], bf16, name="m01", tag="m01", bufs=1)
            nc.vector.tensor_tensor(m01[:, :W], scores[:, :W], zap[:, :W],
                                    AL.is_gt)
            wrt = mkp.tile([128, 2048], bf16, name="wrt", tag="wrt", bufs=1)
            nc.vector.tensor_tensor(wrt[:, :W], scores[:, :W], m01[:, :W],
                                    AL.mult)

            # ---- W^T and m01^T (both PE-transposed) ----
            wtsb = wtp.tile([128, 16, 128], bf16, name="wtsb", tag="wt")
            mtsb = wtp.tile([128, 16, 128], bf16, name="mtsb", tag="mt",
                            bufs=1)
            for src_t, dst in ((wrt, wtsb), (m01, mtsb)):
                for g in range((NT + 7) // 8):
                    n8 = min(8, NT - 8 * g)
                    pw = ps2.tile([128, 8, 128], bf16, name="pw", tag="pswt",
                                  bufs=2)
                    for i in range(n8):
                        st = 8 * g + i
                        nc.tensor.transpose(
                            pw[:, i, :], src_t[:, st * 128:(st + 1) * 128],
                            id_bf)
                    nc.any.tensor_copy(
                        dst[:, 8 * g:8 * g + n8, :].rearrange(
                            "p a c -> p (a c)"),
                        pw[:, :n8, :].rearrange("p a c -> p (a c)"))

            # ---- attention: PV accumulation + denominators fused ----
            # D[h, q] = sum_s exp(l)*m01^T accumulated with a ones-vector
            # matmul into a spare PSUM row of the pair's otp bank (cols
            # 128:256, partition 0 for the even head / 64 for the odd one).
            NG = NT // 4
            for he in range(0, H, 2):
                t6 = he // 2
                otp = psot.tile([128, 256], f32, name="otp", tag="psot")
                Dpair = psD.tile([128, 128], f32, name="Dpair", tag="Dp")
                for h in (he, he + 1):
                    hp = h % 2
                    pb = slice(64 * hp, 64 * hp + 64)
                    drow = Dpair[64 * hp:64 * hp + 1, :]
                    # keys x queries, [s, q] orientation -> P~T -> PV
                    for g in range(NG):
                        lt = ps2.tile([128, 512], f32, name="lt", tag="ps2")
                        ltv = lt.rearrange("p (a c) -> p a c", a=4)
                        for i in range(4):
                            st = 4 * g + i
                            nc.tensor.matmul(
                                ltv[:, i, :],
                                lhsT=KT[pb, t6, st * 128:(st + 1) * 128],
                                rhs=QT[pb, t6, qs])
                        et = etp.tile([128, 512], bf16, name="et", tag="et")
                        nc.scalar.activation(et, lt, AF.Exp, scale=0.125)
                        pt = ptp.tile([128, 512], bf16, name="pt", tag="pt")
                        nc.vector.tensor_tensor(
                            pt, et,
                            wtsb[:, 4 * g:4 * g + 4, :].rearrange(
                                "p a c -> p (a c)"), AL.mult)
                        ptv = pt.rearrange("p (a c) -> p a c", a=4)
                        jk = ptp.tile([128, 512], bf16, name="jk", tag="pt")
                        nc.vector.tensor_tensor(
                            jk, et,
                            mtsb[:, 4 * g:4 * g + 4, :].rearrange(
                                "p a c -> p (a c)"), AL.mult)
                        jkv = jk.rearrange("p (a c) -> p a c", a=4)
                        for i in range(4):
                            st = 4 * g + i
                            nc.tensor.matmul(
                                otp[pb, 0:128],
                                lhsT=Vt[:, st, h * 64:h * 64 + 64],
                                rhs=ptv[:, i, :], start=(st == 0),
                                stop=(st == NT - 1))
                            nc.tensor.matmul(
                                drow, lhsT=ones_bf, rhs=jkv[:, i, :],
                                start=(st == 0), stop=(st == NT - 1))
                if STAGE == "D":
                    dcp = smp.tile([65, 128], f32, name="dcp", tag="dcp", bufs=2)
                    nc.vector.tensor_copy(dcp[0:1, :], Dpair[0:1, :])
                    nc.vector.tensor_copy(dcp[64:65, :], Dpair[64:65, :])
                    nc.sync.dma_start(
                        out=out_d[j * 128 + 2 * t6:j * 128 + 2 * t6 + 1,
                                  0:128],
                        in_=dcp[0:1, :])
                    nc.sync.dma_start(
                        out=out_d[j * 128 + 2 * t6 + 64:
                                  j * 128 + 2 * t6 + 65, 0:128],
                        in_=dcp[64:65, :])
                # D for the pair, bounced via DRAM to broadcast each
                # head's row across the 64 partitions its PV rows occupy;
                # one full-tile reciprocal afterwards (cheap on 128 lanes)
                dD = smp.tile([65, 128], f32, name="dD", tag="ds")
                nc.scalar.copy(dD[0:1, :], Dpair[0:1, :])
                nc.scalar.copy(dD[64:65, :], Dpair[64:65, :])
                dscr0 = ddr.tile([1, 128], f32, name="dscr0", tag="dscr0")
                dscr1 = ddr.tile([1, 128], f32, name="dscr1", tag="dscr1")
                nc.sync.dma_start(out=dscr0, in_=dD[0:1, :])
                nc.sync.dma_start(out=dscr1, in_=dD[64:65, :])
                reps = rpp.tile([128, 128], f32, name="reps", tag="reps",
                                bufs=3)
                nc.sync.dma_start(out=reps[0:64, :],
                                  in_=dscr0.to_broadcast([64, 128]))
                nc.sync.dma_start(out=reps[64:128, :],
                                  in_=dscr1.to_broadcast([64, 128]))
                nc.vector.reciprocal(reps, reps)
                nc.vector.tensor_tensor(otsb[:, t6, qs], otp[:, 0:128],
                                        reps, AL.mult)

            # ---- output projection ----
            if STAGE == "D":
                continue
            for n0, nn in ((0, 512), (512, 256)):
                ops = ps2.tile([128, 512], f32, name="ops", tag="ps2")
                for kt in range(6):
                    nc.tensor.matmul(ops[:, :nn],
                                     lhsT=otsb[:, kt, qs],
                                     rhs=wout_sb[:, kt, n0:n0 + nn],
                                     start=(kt == 0), stop=(kt == 5))
                osb = smp.tile([128, 512], f32, name="osb", tag="osb", bufs=1)
                nc.any.tensor_copy(osb[:, :nn], ops[:, :nn])
                nc.sync.dma_start(out=out_d[qs, n0:n0 + nn], in_=osb[:, :nn])

    _px.close()


# ------------------------------------------------------------------
# host side
# ------------------------------------------------------------------
_CACHE = {}


def _install_ntff_hook():
    """The image lacks antenv.axon_hooks; rebuild it from trn_boot's
    ctypes NTFF profiler so run_bass_kernel_spmd(trace=True) works."""
    import sys
    import types
    if "antenv.axon_hooks" in sys.modules:
        return
    try:
        from trn_agent_boot.trn_boot import _ntff_profile_via_ctypes
        hook = _ntff_profile_via_ctypes("/opt/axon/libaxon_pjrt.so")
    except Exception:
        hook = None
    m = types.ModuleType("antenv.axon_hooks")
    m.get_axon_ntff_profile_hook = lambda: hook
    m.set_axon_ntff_profile_hook = lambda h: None
    sys.modules["antenv.axon_hooks"] = m


def make_inputs_for_core(c, x, wqkvT_bf, wiqT_s, wikT, woutT_bf):
    sm = slot_map(c)
    pbatch = sm[0][0]
    qbatch = sm[1][0]
    xT = [np.ascontiguousarray(x[b].T) for b in range(B)]
    xT_P = xT[pbatch]
    xT_Q = np.ascontiguousarray(xT[qbatch][:, :TQ])
    xTq = np.empty((D, 512), np.float32)
    smask = np.full((128, SMW), FMIN, np.float32)
    for j, (b, r, side) in enumerate(sm):
        xTq[:, j * 128:(j + 1) * 128] = xT[b][:, r * 128:(r + 1) * 128]
        Wj = SLOT_W[j]
        s = np.arange(Wj)[None, :]
        p = np.arange(128)[:, None]
        smask[:, SOFF[j]:SOFF[j] + Wj] = np.where(s <= 128 * r + p, 0.0, FMIN)
    return {
        "xT_P": xT_P, "xT_Q": xT_Q, "xTq": xTq,
        "wqkvT": wqkvT_bf, "wiqT": wiqT_s, "wikT": wikT, "woutT": woutT_bf,
        "smask": smask.astype(ml_dtypes.bfloat16),
    }


def kernel(x, wq_i, bq_i, wk_i, bk_i, w_head, w_qkv, b_qkv, w_out, b_out,
           trace=False):
    x = np.asarray(x, np.float32)
    for b_ in (bq_i, bk_i, b_qkv, b_out):
        assert np.abs(np.asarray(b_)).max() == 0.0, "nonzero bias unsupported"
    w_head = np.asarray(w_head, np.float32)
    signs = tuple(1 if s > 0 else -1 for s in w_head)

    import os
    key = (signs, os.environ.get("KSTAGE", "full"))
    if key not in _CACHE:
        _CACHE[key] = build_program(signs)
    nc = _CACHE[key]

    wqkvT_bf = np.ascontiguousarray(
        np.asarray(w_qkv, np.float32).T).astype(ml_dtypes.bfloat16)
    woutT_bf = np.ascontiguousarray(
        np.asarray(w_out, np.float32).T).astype(ml_dtypes.bfloat16)
    wiq = np.asarray(wq_i, np.float32).reshape(IH, ID, D) * \
        np.abs(w_head)[:, None, None]
    wiqT_s = np.ascontiguousarray(wiq.reshape(IH * ID, D).T)
    wikT = np.ascontiguousarray(np.asarray(wk_i, np.float32).T)

    in_maps = [make_inputs_for_core(c, x, wqkvT_bf, wiqT_s, wikT, woutT_bf)
               for c in range(NCORES)]
    kw = {}
    if trace:
        _install_ntff_hook()
        kw["trace_cores"] = list(range(NCORES))
    res = run_bass_kernel_spmd(nc, in_maps, core_ids=list(range(NCORES)),
                               trace=trace, **kw)

    out = np.empty((B, T, D), np.float32)
    for c in range(NCORES):
        oc = res.results[c]["out"]
        for j, (b, r, _s) in enumerate(slot_map(c)):
            out[b, r * 128:(r + 1) * 128, :] = oc[j * 128:(j + 1) * 128, :]
    kernel.last_result = res
    return out



# revision 33
# speedup vs baseline: 1.0604x; 1.0012x over previous
"""DeepSeek sparse attention (lightning indexer + top-64) on 8 trn2 cores.

Strategy (fully static SPMD program; per-core variation is data-only):
  - Each core owns 4 query tiles of 128 queries ("slots" with fixed key-widths
    512/1024/1536/2048).  Slot -> (batch, qtile) assignment is done on the
    host; each core uploads xT for the two batches it touches ("P" side full
    2048 cols, "Q" side first 1536 cols), the 512 query columns, and an
    additive causal/pad mask per slot.
  - Indexer path (projections + qI.kI scores) runs in fp32 (float32r matmuls)
    so the top-64 selection matches the fp32 reference bit-nearly-exactly.
  - Top-64 per query via 8 rounds of max8 + match_replace; selection becomes
    a 0/1 mask (score > zapped) and routing weights W = score * mask.
  - Attention avoids any gather: logits are computed densely in BOTH
    orientations.  [key, query] orientation: P~T = exp(L/8) * W^T feeds PV
    matmuls directly (masked positions have W == 0).  [query, key]
    orientation feeds exp + masked row-sum (tensor_tensor_reduce) to get the
    softmax denominator.  1/denominator is applied on the tiny PV output.
  - Output projection consumes the feature-major attention output, yielding
    token-major [512, 768] per core, DMA'd straight from PSUM.
"""

import numpy as np
import ml_dtypes

import concourse.bass as bass
import concourse.bacc as bacc
import concourse.mybir as mybir
import concourse.tile as tile
from concourse.bass_utils import run_bass_kernel_spmd

f32 = mybir.dt.float32
f32r = mybir.dt.float32r
bf16 = mybir.dt.bfloat16
AL = mybir.AluOpType
AF = mybir.ActivationFunctionType
AX = mybir.AxisListType

FMIN = -3.0e38
B, T, D = 2, 2048, 768
H, DH, IH, ID, TOPK = 12, 64, 4, 64, 64
TP, TQ = 2048, 1536          # key extents kept for the two batch "sides"
SLOT_W = (512, 1024, 1536, 2048)
SOFF = (0, 512, 1536, 3072)  # smask column offset per slot
SMW = 5120
NCORES = 8


def slot_map(c):
    """slot j -> (batch, qtile_index, side)   side 0 = "P", 1 = "Q"."""
    d, p = c // 2, c % 2
    q = 1 - p
    return [(p, d, 0), (q, 7 - d, 1), (q, 8 + d, 1), (p, 15 - d, 0)]


def _r(ap):
    """[768, C] dram/sbuf view -> [128, 6, C]."""
    return ap.rearrange("(a p) c -> p a c", p=128)


def build_program(signs):
    import os
    _ = os.environ.get("KSTAGE", "full")
    nc = bacc.Bacc("TRN2", target_bir_lowering=False, debug=False,
                   num_devices=NCORES)

    xT_P = nc.dram_tensor("xT_P", [D, TP], f32, kind="ExternalInput").ap()
    xT_Q = nc.dram_tensor("xT_Q", [D, TQ], f32, kind="ExternalInput").ap()
    xTq = nc.dram_tensor("xTq", [D, 512], f32, kind="ExternalInput").ap()
    wqkvT = nc.dram_tensor("wqkvT", [D, 3 * D], bf16, kind="ExternalInput").ap()
    wiqT = nc.dram_tensor("wiqT", [D, IH * ID], f32, kind="ExternalInput").ap()
    wikT = nc.dram_tensor("wikT", [D, IH * ID], f32, kind="ExternalInput").ap()
    woutT = nc.dram_tensor("woutT", [D, D], bf16, kind="ExternalInput").ap()
    smask = nc.dram_tensor("smask", [128, SMW], bf16, kind="ExternalInput").ap()
    out_d = nc.dram_tensor("out", [512, D], f32, kind="ExternalOutput").ap()

    with tile.TileContext(nc) as tc:
        _body(tc, xT_P, xT_Q, xTq, wqkvT, wiqT, wikT, woutT, smask, out_d,
              signs)
    nc.compile()
    return nc


def _body(tc, xT_P, xT_Q, xTq, wqkvT, wiqT, wikT, woutT, smask, out_d, signs):
    nc = tc.nc
    import os
    STAGE = os.environ.get("KSTAGE", "full")

    # ---------------- persistent tensors ----------------
    from contextlib import ExitStack
    _px = ExitStack()
    pp = _px.enter_context(tc.tile_pool(name="persist", bufs=1))
    KT_P = pp.tile([128, 6, TP], bf16, name="KT_P")
    KT_Q = pp.tile([128, 6, TQ], bf16, name="KT_Q")
    V_P = pp.tile([128, TP // 128, D], bf16, name="V_P")
    V_Q = pp.tile([128, TQ // 128, D], bf16, name="V_Q")
    kIT_P = pp.tile([128, 2, TP], f32r, name="kIT_P")
    kIT_Q = pp.tile([128, 2, TQ], f32r, name="kIT_Q")
    QT = pp.tile([128, 6, 512], bf16, name="QT")
    qIT = pp.tile([128, 2, 512], f32r, name="qIT")
    otsb = pp.tile([128, 6, 512], bf16, name="otsb")
    id_bf = pp.tile([128, 128], bf16, name="id_bf")
    ones_bf = pp.tile([128, 1], bf16, name="ones_bf")

    # identity matrix for PE transposes
    nc.vector.memset(id_bf, 1.0)
    nc.gpsimd.affine_select(id_bf, id_bf, pattern=[[-1, 128]],
                            compare_op=AL.is_equal, fill=0.0, base=0,
                            channel_multiplier=1)
    nc.vector.memset(ones_bf, 1.0)

    # ---------------- stage 1: projections ----------------
    CH = 256
    with tc.tile_pool(name="w1", bufs=1) as wp, \
         tc.tile_pool(name="ps1", bufs=3, space="PSUM") as ps1, \
         tc.tile_pool(name="psv", bufs=2, space="PSUM") as psv:
        wqkv_sb = wp.tile([128, 6, 3 * D], bf16)
        wiq_sb = wp.tile([128, 6, IH * ID], f32)
        wik_sb = wp.tile([128, 6, IH * ID], f32)
        wqv = _r(wqkvT)
        for a in range(6):
            nc.sync.dma_start(out=wqkv_sb[:, a, :], in_=wqv[:, a, :])
        wiv = _r(wiqT)
        wkv = _r(wikT)
        for a in range(0, 6, 3):
            nc.sync.dma_start(out=wiq_sb[:, a:a + 3, :],
                              in_=wiv[:, a:a + 3, :])
            nc.sync.dma_start(out=wik_sb[:, a:a + 3, :],
                              in_=wkv[:, a:a + 3, :])
        # fp32r operands must come from an op that rounds to fp32r
        wiq_r = wp.tile([128, 6, IH * ID], f32r)
        wik_r = wp.tile([128, 6, IH * ID], f32r)
        nc.any.tensor_copy(wiq_r.rearrange("p a c -> p (a c)"),
                           wiq_sb.rearrange("p a c -> p (a c)"))
        nc.any.tensor_copy(wik_r.rearrange("p a c -> p (a c)"),
                           wik_sb.rearrange("p a c -> p (a c)"))

        # queries first (they are needed by every slot)
        with tc.tile_pool(name="xq", bufs=1) as xqp:
            xqv = _r(xTq)
            for q0 in range(0, 512, 256):
                xq = xqp.tile([128, 6, 256], f32, name="xq", tag="xq")
                for a in range(0, 6, 3):
                    nc.sync.dma_start(out=xq[:, a:a + 3, :],
                                      in_=xqv[:, a:a + 3, q0:q0 + 256])
                xqb = xqp.tile([128, 6, 256], bf16, name="xqb", tag="xqb")
                nc.any.tensor_copy(xqb.rearrange("p a c -> p (a c)"),
                                   xq.rearrange("p a c -> p (a c)"))
                xqr = xqp.tile([128, 6, 256], f32r, name="xqr", tag="xqr")
                nc.any.tensor_copy(xqr.rearrange("p a c -> p (a c)"),
                                   xq.rearrange("p a c -> p (a c)"))
                for mt in range(2):
                    ps = ps1.tile([128, 256], f32, name="ps_qi", tag="ps1")
                    for kt in range(6):
                        nc.tensor.matmul(
                            ps, lhsT=wiq_r[:, kt, mt * 128:(mt + 1) * 128],
                            rhs=xqr[:, kt, :], start=(kt == 0), stop=(kt == 5))
                    nc.any.tensor_copy(qIT[:, mt, q0:q0 + 256], ps)
                for mt in range(6):
                    ps = ps1.tile([128, 256], f32, name="ps_qt", tag="ps1")
                    for kt in range(6):
                        nc.tensor.matmul(
                            ps, lhsT=wqkv_sb[:, kt, mt * 128:(mt + 1) * 128],
                            rhs=xqb[:, kt, :], start=(kt == 0), stop=(kt == 5))
                    nc.any.tensor_copy(QT[:, mt, q0:q0 + 256], ps)

        with tc.tile_pool(name="xc", bufs=2) as xcp, \
             tc.tile_pool(name="xcb", bufs=2) as xbp:
            for side, (xT, S) in enumerate(((xT_P, TP), (xT_Q, TQ))):
                xv = _r(xT)
                KT = KT_P if side == 0 else KT_Q
                Vt = V_P if side == 0 else V_Q
                kIT = kIT_P if side == 0 else kIT_Q
                for c0 in range(0, S, CH):
                    xc = xcp.tile([128, 6, CH], f32, name="xc", tag="xc")
                    for a in range(0, 6, 3):
                        nc.sync.dma_start(out=xc[:, a:a + 3, :],
                                          in_=xv[:, a:a + 3, c0:c0 + CH])
                    xcb = xbp.tile([128, 6, CH], bf16, name="xcb", tag="xcb")
                    nc.any.tensor_copy(xcb.rearrange("p a c -> p (a c)"),
                                       xc.rearrange("p a c -> p (a c)"))
                    xcr = xbp.tile([128, 6, CH], f32r, name="xcr", tag="xcr", bufs=1)
                    nc.any.tensor_copy(xcr.rearrange("p a c -> p (a c)"),
                                       xc.rearrange("p a c -> p (a c)"))
                    # indexer keys (fp32r, exact)
                    for mt in range(2):
                        ps = ps1.tile([128, CH], f32, name="ps_ki", tag="ps1")
                        for kt in range(6):
                            nc.tensor.matmul(
                                ps, lhsT=wik_r[:, kt, mt * 128:(mt + 1) * 128],
                                rhs=xcr[:, kt, :],
                                start=(kt == 0), stop=(kt == 5))
                        nc.any.tensor_copy(kIT[:, mt, c0:c0 + CH], ps)
                    # attention keys KT (bf16)
                    for mt in range(6):
                        ps = ps1.tile([128, CH], f32, name="ps_kt", tag="ps1")
                        for kt in range(6):
                            nc.tensor.matmul(
                                ps,
                                lhsT=wqkv_sb[:, kt,
                                             D + mt * 128:D + (mt + 1) * 128],
                                rhs=xcb[:, kt, :], start=(kt == 0),
                                stop=(kt == 5))
                        nc.any.tensor_copy(KT[:, mt, c0:c0 + CH], ps)
                    # values V (token-major, bf16)
                    for st in range(CH // 128):
                        ps = psv.tile([128, D], f32, name="ps_v", tag="psv")
                        for n0, nn in ((0, 512), (512, 256)):
                            for kt in range(6):
                                nc.tensor.matmul(
                                    ps[:, n0:n0 + nn],
                                    lhsT=xcb[:, kt, st * 128:(st + 1) * 128],
                                    rhs=wqkv_sb[:, kt,
                                                2 * D + n0:2 * D + n0 + nn],
                                    start=(kt == 0), stop=(kt == 5))
                        nc.any.tensor_copy(Vt[:, c0 // 128 + st, :], ps)

    # ---------------- stage 2/3: per query tile ----------------
    with tc.tile_pool(name="w2", bufs=1) as wp2, \
         tc.tile_pool(name="sc", bufs=2) as scp, \
         tc.tile_pool(name="zap", bufs=1) as zpp, \
         tc.tile_pool(name="msk", bufs=2) as mkp, \
         tc.tile_pool(name="wts", bufs=2) as wtp, \
         tc.tile_pool(name="u", bufs=2) as upl, \
         tc.tile_pool(name="et", bufs=2) as etp, \
         tc.tile_pool(name="pt", bufs=4) as ptp, \
         tc.tile_pool(name="sm", bufs=4) as smp, \
         tc.tile_pool(name="rp", bufs=1) as rpp, \
         tc.tile_pool(name="ps2", bufs=2, space="PSUM") as ps2, \
         tc.tile_pool(name="psot", bufs=2, space="PSUM") as psot, \
         tc.tile_pool(name="psD", bufs=2, space="PSUM") as psD, \
         tc.tile_pool(name="ddr", bufs=2, space="DRAM") as ddr:
        wout_sb = wp2.tile([128, 6, D], bf16)
        wov = _r(woutT)
        for a in range(0, 6, 3):
            nc.sync.dma_start(out=wout_sb[:, a:a + 3, :],
                              in_=wov[:, a:a + 3, :])
        smask_sb = wp2.tile([128, SMW], bf16)
        nc.sync.dma_start(out=smask_sb, in_=smask)

        for j in range(4):
            W = SLOT_W[j]
            side = (0, 1, 1, 0)[j]
            KT = KT_P if side == 0 else KT_Q
            Vt = V_P if side == 0 else V_Q
            kIT = kIT_P if side == 0 else kIT_Q
            NT = W // 128
            NCH = W // 512
            qs = slice(j * 128, (j + 1) * 128)

            # ---- indexer scores ----
            scores = scp.tile([128, 2048], f32, name="scores", tag="scores")
            for ch in range(NCH):
                cs = slice(ch * 512, (ch + 1) * 512)
                us = []
                for h4 in range(IH):
                    hp, t2 = h4 % 2, h4 // 2
                    ps = ps2.tile([128, 512], f32, name="ps_ix", tag="ps2")
                    nc.tensor.matmul(
                        ps, lhsT=qIT[64 * hp:64 * hp + 64, t2, qs],
                        rhs=kIT[64 * hp:64 * hp + 64, t2, cs])
                    uh = upl.tile([128, 512], f32, name="u", tag="u")
                    nc.scalar.activation(uh, ps, AF.Relu)
                    us.append(uh)
                # signed head-sum + causal/pad mask, all on gpsimd:
                # sc = smask +- u0 +- u1 +- u2 +- u3  (|w_h| folded into qIT)
                smk = smask_sb[:, SOFF[j] + ch * 512:SOFF[j] + (ch + 1) * 512]
                sc = scores[:, cs]
                if signs[0] > 0:
                    nc.gpsimd.tensor_tensor(sc, us[0], smk, AL.add)
                else:
                    nc.gpsimd.tensor_tensor(sc, smk, us[0], AL.subtract)
                for h4 in range(1, IH):
                    nc.gpsimd.tensor_tensor(
                        sc, sc, us[h4],
                        AL.add if signs[h4] > 0 else AL.subtract)

            # ---- top-64 threshold via 8x (max8 + match_replace) ----
            zap = zpp.tile([128, 2048], f32, name="zap", tag="zap")
            src = scores
            for r in range(8):
                m8 = smp.tile([128, 8], f32, name="m8", tag="m8")
                nc.vector.max(out=m8, in_=src[:, :W])
                nc.vector.match_replace(out=zap[:, :W], in_to_replace=m8,
                                        in_values=src[:, :W], imm_value=FMIN)
                src = zap
            # Masked positions carry smask(bf16)+score, not exactly FMIN; a
            # replaced masked slot would compare greater than FMIN and get
            # selected.  Clamping the zapped scores to -1e38 (far above any
            # masked value, far below any real score) kills those.
            nc.vector.tensor_scalar(zap[:, :W], zap[:, :W], -1.0e38, None,
                                    AL.max)

            m01 = mkp.tile([128, 2048], bf16, name="m01", tag="m01", bufs=1)
            nc.vector.tensor_tensor(m01[:, :W], scores[:, :W], zap[:, :W],
                                    AL.is_gt)
            wrt = mkp.tile([128, 2048], bf16, name="wrt", tag="wrt", bufs=1)
            nc.vector.tensor_tensor(wrt[:, :W], scores[:, :W], m01[:, :W],
                                    AL.mult)

            # ---- W^T and m01^T (both PE-transposed) ----
            wtsb = wtp.tile([128, 16, 128], bf16, name="wtsb", tag="wt")
            mtsb = wtp.tile([128, 16, 128], bf16, name="mtsb", tag="mt",
                            bufs=1)
            for src_t, dst in ((wrt, wtsb), (m01, mtsb)):
                for g in range((NT + 7) // 8):
                    n8 = min(8, NT - 8 * g)
                    pw = ps2.tile([128, 8, 128], bf16, name="pw", tag="pswt",
                                  bufs=2)
                    for i in range(n8):
                        st = 8 * g + i
                        nc.tensor.transpose(
                            pw[:, i, :], src_t[:, st * 128:(st + 1) * 128],
                            id_bf)
                    nc.any.tensor_copy(
                        dst[:, 8 * g:8 * g + n8, :].rearrange(
                            "p a c -> p (a c)"),
                        pw[:, :n8, :].rearrange("p a c -> p (a c)"))

            # ---- attention: PV accumulation + denominators fused ----
            # D[h, q] = sum_s exp(l)*m01^T accumulated with a ones-vector
            # matmul into a spare PSUM row of the pair's otp bank (cols
            # 128:256, partition 0 for the even head / 64 for the odd one).
            NG = NT // 4
            for he in range(0, H, 2):
                t6 = he // 2
                otp = psot.tile([128, 256], f32, name="otp", tag="psot")
                Dpair = psD.tile([128, 128], f32, name="Dpair", tag="Dp")
                for h in (he, he + 1):
                    hp = h % 2
                    pb = slice(64 * hp, 64 * hp + 64)
                    drow = Dpair[64 * hp:64 * hp + 1, :]
                    # keys x queries, [s, q] orientation -> P~T -> PV
                    for g in range(NG):
                        lt = ps2.tile([128, 512], f32, name="lt", tag="ps2")
                        ltv = lt.rearrange("p (a c) -> p a c", a=4)
                        for i in range(4):
                            st = 4 * g + i
                            nc.tensor.matmul(
                                ltv[:, i, :],
                                lhsT=KT[pb, t6, st * 128:(st + 1) * 128],
                                rhs=QT[pb, t6, qs])
                        et = etp.tile([128, 512], bf16, name="et", tag="et")
                        nc.scalar.activation(et, lt, AF.Exp, scale=0.125)
                        pt = ptp.tile([128, 512], bf16, name="pt", tag="pt")
                        nc.vector.tensor_tensor(
                            pt, et,
                            wtsb[:, 4 * g:4 * g + 4, :].rearrange(
                                "p a c -> p (a c)"), AL.mult)
                        ptv = pt.rearrange("p (a c) -> p a c", a=4)
                        jk = ptp.tile([128, 512], bf16, name="jk", tag="pt")
                        nc.vector.tensor_tensor(
                            jk, et,
                            mtsb[:, 4 * g:4 * g + 4, :].rearrange(
                                "p a c -> p (a c)"), AL.mult)
                        jkv = jk.rearrange("p (a c) -> p a c", a=4)
                        for i in range(4):
                            st = 4 * g + i
                            nc.tensor.matmul(
                                otp[pb, 0:128],
                                lhsT=Vt[:, st, h * 64:h * 64 + 64],
                                rhs=ptv[:, i, :], start=(st == 0),
                                stop=(st == NT - 1))
                            nc.tensor.matmul(
                                drow, lhsT=ones_bf, rhs=jkv[:, i, :],
                                start=(st == 0), stop=(st == NT - 1))
                if STAGE == "D":
                    dcp = smp.tile([65, 128], f32, name="dcp", tag="dcp", bufs=2)
                    nc.vector.tensor_copy(dcp[0:1, :], Dpair[0:1, :])
                    nc.vector.tensor_copy(dcp[64:65, :], Dpair[64:65, :])
                    nc.sync.dma_start(
                        out=out_d[j * 128 + 2 * t6:j * 128 + 2 * t6 + 1,
                                  0:128],
                        in_=dcp[0:1, :])
                    nc.sync.dma_start(
                        out=out_d[j * 128 + 2 * t6 + 64:
                                  j * 128 + 2 * t6 + 65, 0:128],
                        in_=dcp[64:65, :])
                # D for the pair, bounced via DRAM to broadcast each
                # head's row across the 64 partitions its PV rows occupy;
                # one full-tile reciprocal afterwards (cheap on 128 lanes)
                dD = smp.tile([65, 128], f32, name="dD", tag="ds")
                nc.scalar.copy(dD[0:1, :], Dpair[0:1, :])
                nc.scalar.copy(dD[64:65, :], Dpair[64:65, :])
                dscr0 = ddr.tile([1, 128], f32, name="dscr0", tag="dscr0")
                dscr1 = ddr.tile([1, 128], f32, name="dscr1", tag="dscr1")
                nc.sync.dma_start(out=dscr0, in_=dD[0:1, :])
                nc.sync.dma_start(out=dscr1, in_=dD[64:65, :])
                reps = rpp.tile([128, 128], f32, name="reps", tag="reps",
                                bufs=3)
                nc.sync.dma_start(out=reps[0:64, :],
                                  in_=dscr0.to_broadcast([64, 128]))
                nc.sync.dma_start(out=reps[64:128, :],
                                  in_=dscr1.to_broadcast([64, 128]))
                nc.vector.reciprocal(reps, reps)
                nc.vector.tensor_tensor(otsb[:, t6, qs], otp[:, 0:128],
                                        reps, AL.mult)

            # ---- output projection ----
            if STAGE == "D":
                continue
            for n0, nn in ((0, 512), (512, 256)):
                ops = ps2.tile([128, 512], f32, name="ops", tag="ps2")
                for kt in range(6):
                    nc.tensor.matmul(ops[:, :nn],
                                     lhsT=otsb[:, kt, qs],
                                     rhs=wout_sb[:, kt, n0:n0 + nn],
                                     start=(kt == 0), stop=(kt == 5))
                osb = smp.tile([128, 512], f32, name="osb", tag="osb", bufs=1)
                nc.any.tensor_copy(osb[:, :nn], ops[:, :nn])
                nc.sync.dma_start(out=out_d[qs, n0:n0 + nn], in_=osb[:, :nn])

    _px.close()


# ------------------------------------------------------------------
# host side
# ------------------------------------------------------------------
_CACHE = {}


def _install_ntff_hook():
    """The image lacks antenv.axon_hooks; rebuild it from trn_boot's
    ctypes NTFF profiler so run_bass_kernel_spmd(trace=True) works."""
    import sys
    import types
    if "antenv.axon_hooks" in sys.modules:
        return
    try:
        from trn_agent_boot.trn_boot import _ntff_profile_via_ctypes
        hook = _ntff_profile_via_ctypes("/opt/axon/libaxon_pjrt.so")
    except Exception:
        hook = None
    m = types.ModuleType("antenv.axon_hooks")
    m.get_axon_ntff_profile_hook = lambda: hook
    m.set_axon_ntff_profile_hook = lambda h: None
    sys.modules["antenv.axon_hooks"] = m


def make_inputs_for_core(c, x, wqkvT_bf, wiqT_s, wikT, woutT_bf):
    sm = slot_map(c)
    pbatch = sm[0][0]
    qbatch = sm[1][0]
    xT = [np.ascontiguousarray(x[b].T) for b in range(B)]
    xT_P = xT[pbatch]
    xT_Q = np.ascontiguousarray(xT[qbatch][:, :TQ])
    xTq = np.empty((D, 512), np.float32)
    smask = np.full((128, SMW), FMIN, np.float32)
    for j, (b, r, side) in enumerate(sm):
        xTq[:, j * 128:(j + 1) * 128] = xT[b][:, r * 128:(r + 1) * 128]
        Wj = SLOT_W[j]
        s = np.arange(Wj)[None, :]
        p = np.arange(128)[:, None]
        smask[:, SOFF[j]:SOFF[j] + Wj] = np.where(s <= 128 * r + p, 0.0, FMIN)
    return {
        "xT_P": xT_P, "xT_Q": xT_Q, "xTq": xTq,
        "wqkvT": wqkvT_bf, "wiqT": wiqT_s, "wikT": wikT, "woutT": woutT_bf,
        "smask": smask.astype(ml_dtypes.bfloat16),
    }


def kernel(x, wq_i, bq_i, wk_i, bk_i, w_head, w_qkv, b_qkv, w_out, b_out,
           trace=False):
    x = np.asarray(x, np.float32)
    for b_ in (bq_i, bk_i, b_qkv, b_out):
        assert np.abs(np.asarray(b_)).max() == 0.0, "nonzero bias unsupported"
    w_head = np.asarray(w_head, np.float32)
    signs = tuple(1 if s > 0 else -1 for s in w_head)

    import os
    key = (signs, os.environ.get("KSTAGE", "full"))
    if key not in _CACHE:
        _CACHE[key] = build_program(signs)
    nc = _CACHE[key]

    wqkvT_bf = np.ascontiguousarray(
        np.asarray(w_qkv, np.float32).T).astype(ml_dtypes.bfloat16)
    woutT_bf = np.ascontiguousarray(
        np.asarray(w_out, np.float32).T).astype(ml_dtypes.bfloat16)
    wiq = np.asarray(wq_i, np.float32).reshape(IH, ID, D) * \
        np.abs(w_head)[:, None, None]
    wiqT_s = np.ascontiguousarray(wiq.reshape(IH * ID, D).T)
    wikT = np.ascontiguousarray(np.asarray(wk_i, np.float32).T)

    in_maps = [make_inputs_for_core(c, x, wqkvT_bf, wiqT_s, wikT, woutT_bf)
               for c in range(NCORES)]
    kw = {}
    if trace:
        _install_ntff_hook()
        kw["trace_cores"] = list(range(NCORES))
    res = run_bass_kernel_spmd(nc, in_maps, core_ids=list(range(NCORES)),
                               trace=trace, **kw)

    out = np.empty((B, T, D), np.float32)
    for c in range(NCORES):
        oc = res.results[c]["out"]
        for j, (b, r, _s) in enumerate(slot_map(c)):
            out[b, r * 128:(r + 1) * 128, :] = oc[j * 128:(j + 1) * 128, :]
    kernel.last_result = res
    return out

